# revision 37
# baseline (speedup 1.0000x reference)
"""Trainium2 Bass kernel for nn_AutoCorrelation (Autoformer AutoCorrelation).

Math identical to the validated baseline (dense rfft-as-matmul with radix-2
time fold, DVE top-8, mod-64 roll as per-head 64x64 matmuls, scrambling
reshape folded into the final projection's lhsT layout). All matmul data
stays float32r (bf16 anywhere in the Q/K correlation path measurably breaks
the top-k selection: numpy study gives rel err 1.2e-2..3.4e-2 vs 5e-4).

Restructured for PE throughput vs the first working version (1.10 ms -> ~0.66 ms
neuron-profile device exec):
  - q/k loaded + transposed ONCE (the old version re-did both per channel
    group): the radix-2 time fold now happens on the RAW inputs (DVE adds,
    fold commutes with the projection), so folded spectra inputs come
    straight out of the projection and the PSUM fold drain disappears.
  - Q spectra land in SBUF; K spectra stay in PSUM and the spectral product
    overwrites the Q spectra in place (saves 32 PSUM->SBUF copies and all
    pool-lifetime conflicts nest LIFO as the tile framework requires).
  - The V path runs FIRST (it is independent): it warms the PE before the
    projections and parks projected V^T in DRAM, re-read at the roll phase,
    which frees 64KB/partition through the F/G/top-k phases.
  - PSUM drains alternate DVE/scalar; G-combine partner terms for odd t go
    to gpsimd; top-k(0) chunks are emitted between G(g1) quadrants so the
    DVE queue never blocks the G combines the PE waits on.
  - G accumulates the odd-k tiles first so their PSUM banks drain to SBUF
    while the even-k tiles accumulate, keeping all 8 banks cycling.

Sharding: data-parallel over batch B=8 -> one batch element per NeuronCore.
"""
import numpy as np

B, L, D, H, Dh = 8, 2048, 1024, 16, 64
KTOP = 7
NCORES = 8
P = 128

_prog_cache = {}


# ---------------------------------------------------------------- host helpers
def _round_fp32r(x):
    """Round fp32 to the ~13-bit-mantissa fp32r grid (matches measured HW)."""
    b = np.ascontiguousarray(x, dtype=np.float32).view(np.uint32)
    half = np.uint32(1 << 9)
    keep = np.uint32(0xFFFFFFFF) ^ np.uint32((1 << 10) - 1)
    out = ((b + half) & keep).view(np.float32)
    return np.where(np.isfinite(out), out, 0.0).astype(np.float32)


def _k_of(j, r):
    """k-index of packed row/col 128*j+r in the parity-permuted layout."""
    if j < 4:
        return 2 * (128 * j + r), "re"
    if j < 8:
        return 2 * (128 * (j - 4) + r) + 1, "re"
    if j < 12:
        return 2 * (128 * (j - 8) + r), "im"
    return 2 * (128 * (j - 12) + r) + 1, "im"


def _build_FG():
    """Radix-2-folded DFT matrices in the parity-permuted packed layout.

    Fh (1024, 2048): cols for even-k tiles multiply Qe = q[:1024]+q[1024:],
    odd-k tiles multiply Qo = q[:1024]-q[1024:]; sqrt(c_k/L) folded in.
    G (2048, 1024): rows give n in [0,1024); corr[n+1024] is recovered from
    the even-tile/odd-tile partial sums as A - B. Tile 8 row 0 carries the
    nyquist term (its natural occupant, im k=0, is identically zero)."""
    m = np.arange(1024)[:, None].astype(np.float64)
    n = np.arange(1024)[None, :].astype(np.float64)
    Fh = np.zeros((1024, 2048), dtype=np.float64)
    G = np.zeros((2048, 1024), dtype=np.float64)
    for j in range(16):
        for r in range(128):
            k, ri = _k_of(j, r)
            col = 128 * j + r
            if j == 8 and r == 0:  # nyquist slot
                Fh[:, col] = (np.cos(np.pi * m[:, 0]) * np.sqrt(1.0 / L))
                G[col, :] = np.cos(np.pi * n[0])
                continue
            ck = 1.0 if k == 0 else 2.0
            if ri == "re":
                Fh[:, col] = np.cos(2 * np.pi * m[:, 0] * k / L) * np.sqrt(ck / L)
                G[col, :] = np.cos(2 * np.pi * k * n[0] / L)
            else:
                Fh[:, col] = -np.sin(2 * np.pi * m[:, 0] * k / L) * np.sqrt(ck / L)
                G[col, :] = -np.sin(2 * np.pi * k * n[0] / L)
    return Fh.astype(np.float32), G.astype(np.float32)


def _tile_w(w):
    """(1024, 1024) -> (128, 8, 1024): [p, o, c] = w[o*128+p, c]."""
    return np.ascontiguousarray(
        w.reshape(8, P, D).transpose(1, 0, 2)).astype(np.float32)


def _host_constants():
    Fh, G = _build_FG()
    # fmat[t, p, mt] = [Fh[mt*P+p, t-block] | Fh[mt*P+p, (8+t)-block]]
    # (p-major so the per-t DMA into a [128, 8, 256] tile is contiguous)
    fmat = np.zeros((8, P, 8, 2 * P), dtype=np.float32)
    for t in range(8):
        for mt in range(8):
            fmat[t, :, mt, 0:P] = Fh[mt * P:(mt + 1) * P, t * P:(t + 1) * P]
            fmat[t, :, mt, P:2 * P] = Fh[mt * P:(mt + 1) * P,
                                         (8 + t) * P:(9 + t) * P]
    # gmat[kt, nn] = G[kt*128:(kt+1)*128, nn*512:(nn+1)*512], nn in 0..1
    gmat = np.zeros((16, 2, P, 512), dtype=np.float32)
    for kt in range(16):
        for nn in range(2):
            gmat[kt, nn] = G[kt * P:(kt + 1) * P, nn * 512:(nn + 1) * 512]
    return _round_fp32r(fmat), _round_fp32r(gmat)


# ---------------------------------------------------------------- wait splitting
def _split_excess_waits(nc, mybir):
    """walrus here allows max ONE sem wait per instruction; hoist extras onto
    injected NoOps on the same engine."""
    ctr = 0
    for f in nc.m.functions:
        for bblk in f.blocks:
            insts = bblk.instructions
            i = 0
            while i < len(insts):
                ins = insts[i]
                si = ins.sync_info
                if si is not None and len(si.on_wait) > 1:
                    waits = list(si.on_wait)
                    for w in waits[:-1]:
                        nop = mybir.InstNoOp(name=f"WSPL-{ctr}", ins=[], outs=[])
                        ctr += 1
                        nop.engine = ins.engine
                        nop.sync_info = mybir.SyncInfo(on_wait=[w], on_update=[])
                        insts.insert(i, nop)
                        i += 1
                    ins.sync_info = mybir.SyncInfo(
                        on_wait=[waits[-1]], on_update=list(si.on_update))
                i += 1


# ---------------------------------------------------------------- bass program
def _build_program():
    import concourse.bass as bass
    import concourse.mybir as mybir
    from concourse.tile import TileContext
    from concourse.masks import make_identity

    f32 = mybir.dt.float32
    f32r = mybir.dt.float32r
    i32 = mybir.dt.int32
    u32 = mybir.dt.uint32
    Alu = mybir.AluOpType

    nc = bass.Bass("TRN2", target_bir_lowering=False)

    qin = nc.dram_tensor("qin", (L, D), f32, kind="ExternalInput")
    kin = nc.dram_tensor("kin", (L, D), f32, kind="ExternalInput")
    vin = nc.dram_tensor("vin", (L, D), f32, kind="ExternalInput")
    wq_d = nc.dram_tensor("wq_t", (P, 8, D), f32r, kind="ExternalInput")
    wk_d = nc.dram_tensor("wk_t", (P, 8, D), f32r, kind="ExternalInput")
    wv_d = nc.dram_tensor("wv_t", (P, 8, D), f32r, kind="ExternalInput")
    wo_d = nc.dram_tensor("wo_t", (P, 8, D), f32r, kind="ExternalInput")
    f_d = nc.dram_tensor("fmat", (8, P, 8, 2 * P), f32r, kind="ExternalInput")
    g_d = nc.dram_tensor("gmat", (16, 2, P, 512), f32r, kind="ExternalInput")
    out_d = nc.dram_tensor("out", (L, D), f32, kind="ExternalOutput")
    vp_d = nc.dram_tensor("vproj_dram", (P, 8, L), f32r, kind="Internal")
    taps = {}
    if _prog_cache.get("debug_taps"):
        taps["feq"] = nc.dram_tensor("tap_feq", (2, P, 8, 512), f32r,
                                     kind="ExternalOutput")
        taps["foq"] = nc.dram_tensor("tap_foq", (2, P, 8, 512), f32r,
                                     kind="ExternalOutput")
        taps["sq"] = nc.dram_tensor("tap_sq", (2, P, 16, 512), f32r,
                                    kind="ExternalOutput")
        taps["sp"] = nc.dram_tensor("tap_sp", (2, P, 16, 512), f32r,
                                    kind="ExternalOutput")
        taps["corr"] = nc.dram_tensor("tap_corr", (2, P, 4, L), f32,
                                      kind="ExternalOutput")
        taps["m"] = nc.dram_tensor("tap_m", (P, 8, 64), f32,
                                   kind="ExternalOutput")
        taps["vp"] = nc.dram_tensor("tap_vp", (P, 8, L), f32,
                                    kind="ExternalOutput")
        taps["r2t"] = nc.dram_tensor("tap_r2t", (P, 8, 64, 32), f32r,
                                     kind="ExternalOutput")

    _rr = [0]

    def drain(dst, src_ap):
        # PSUM -> SBUF copy. f32r destinations must go through the DVE
        # (scalar/activation f32r output is not trustworthy); plain f32
        # copies alternate DVE/scalar to spread load.
        if _rr[0] % 2 == 0:
            nc.vector.tensor_copy(dst, src_ap)
        else:
            nc.scalar.copy(dst, src_ap)
        _rr[0] += 1

    with TileContext(nc) as tc:
        with tc.tile_pool(name="const", bufs=1) as cp:
            ident = cp.tile([P, P], f32)
            make_identity(nc, ident)
            # (128, 64) with 1 at (p, p % 64): a 64x64 identity at base 0 or 64
            ident2 = cp.tile([P, 64], f32)
            nc.gpsimd.memset(ident2, 0.0)
            nc.gpsimd.affine_select(
                out=ident2, in_=ident2, compare_op=Alu.not_equal, fill=1.0,
                base=0, channel_multiplier=1, pattern=[[-1, 64]])
            nc.gpsimd.affine_select(
                out=ident2, in_=ident2, compare_op=Alu.not_equal, fill=1.0,
                base=-64, channel_multiplier=1, pattern=[[-1, 64]])
            # T64f[p, s] = (p - s) & 63 as fp32
            t64i = cp.tile([P, 64], i32)
            nc.gpsimd.iota(t64i, pattern=[[-1, 64]], base=0, channel_multiplier=1)
            nc.vector.tensor_scalar(t64i, t64i, 63, None, op0=Alu.bitwise_and)
            t64f = cp.tile([P, 64], f32)
            nc.vector.tensor_copy(t64f, t64i)
            m_all = cp.tile([P, 8, 64], f32)    # roll matrices, (c, s)
            mp_all = cp.tile([P, 8, 64], f32r)  # transposed,  (s, t) per head
            smf = cp.tile([P, 8, 32], f32)      # per-cc top-k scratch
            smu = cp.tile([P, 8, 16], u32)
            tmp64a = cp.tile([P, 64], f32)
            tmp64b = cp.tile([P, 64], f32)

            # ------------- phase P/F: per tensor: fold -> transpose ->
            # project -> spectra.  Q spectra land in s_g; K spectra stay in
            # PSUM and the spectral product overwrites s_g in place.
            def proj_tensor(name, src_d, w_d):
                fep = tc.alloc_tile_pool(name=f"fe{name}", bufs=1)
                fe = [fep.tile([P, 8, 512], f32r, name=f"fe{name}{g}")
                      for g in range(2)]
                fo = [fep.tile([P, 8, 512], f32r, name=f"fo{name}{g}")
                      for g in range(2)]
                wpool = tc.alloc_tile_pool(name=f"w{name}", bufs=1)
                w_sb = wpool.tile([P, 8, D], f32r, name=f"w_{name}")
                for cc in range(8):
                    nc.sync.dma_start(w_sb[:, cc, :], w_d.ap()[:, cc, :])
                with tc.tile_pool(name=f"raw{name}", bufs=2) as rawp, \
                     tc.tile_pool(name=f"eo{name}", bufs=2) as eop, \
                     tc.tile_pool(name=f"xt{name}", bufs=3) as xtp, \
                     tc.tile_pool(name=f"trp{name}", bufs=2,
                                  space="PSUM") as trpp, \
                     tc.tile_pool(name=f"pp{name}", bufs=3,
                                  space="PSUM") as prjp:
                    for mc in range(8):
                        rlo = rawp.tile([P, D], f32, tag="rlo")
                        rhi = rawp.tile([P, D], f32, tag="rhi")
                        nc.sync.dma_start(rlo, src_d.ap()[mc * P:(mc + 1) * P, :])
                        nc.sync.dma_start(
                            rhi, src_d.ap()[(mc + 8) * P:(mc + 9) * P, :])
                        for br, op in ((0, Alu.add), (1, Alu.subtract)):
                            eo = eop.tile([P, D], f32, tag="eo",
                                          name=f"eo_{name}{mc}{br}")
                            nc.vector.tensor_tensor(eo, rlo, rhi, op)
                            # transpose eo -> xt (c-part, m) then project
                            xt = xtp.tile([P, 8, P], f32r, tag="xt",
                                          name=f"xt_{name}{mc}{br}")
                            for half in range(2):
                                trp = trpp.tile([P, 512], f32, tag="tr")
                                for j in range(4):
                                    ct = half * 4 + j
                                    nc.tensor.transpose(
                                        trp[:, j * P:(j + 1) * P],
                                        eo[:, ct * P:(ct + 1) * P], ident)
                                drain(xt[:, half * 4:(half + 1) * 4, :], trp)
                            dst = fe if br == 0 else fo
                            pr = prjp.tile([P, 2, 512], f32, tag="pr",
                                           name=f"pr_{name}{mc}{br}")
                            for g in range(2):
                                for cc in range(8):
                                    nc.tensor.matmul(
                                        pr[:, g, :], xt[:, cc, :],
                                        w_sb[:, cc, g * 512:(g + 1) * 512],
                                        start=(cc == 0), stop=(cc == 7))
                                drain(dst[g][:, mc, :], pr[:, g, :])
                wpool.release()
                return fep, fe, fo

            # ------------- phase V first (independent; warms the PE);
            # projected V^T goes to DRAM and is re-read at the roll phase
            wv_pool = tc.alloc_tile_pool(name="wv", bufs=1)
            wv_sb = wv_pool.tile([P, 8, D], f32r)
            with tc.tile_pool(name="vraw", bufs=2) as vrawp, \
                 tc.tile_pool(name="vxt", bufs=2) as vxtp, \
                 tc.tile_pool(name="vst", bufs=4) as vstp, \
                 tc.tile_pool(name="vtrp", bufs=2, space="PSUM") as vtrpp, \
                 tc.tile_pool(name="vpp", bufs=4, space="PSUM") as vpp:
                vxts = [None] * 4

                def v_transpose(mb):
                    vxt = vxtp.tile([P, 8, 512], f32r, tag="vxt",
                                    name=f"vxt_{mb}")
                    vxts[mb] = vxt
                    for mc in range(4):
                        raw = vrawp.tile([P, D], f32, tag="vraw")
                        nc.sync.dma_start(
                            raw, vin.ap()[(mb * 4 + mc) * P:
                                          (mb * 4 + mc + 1) * P, :])
                        for half in range(2):
                            trp = vtrpp.tile([P, 512], f32, tag="vtr")
                            for j in range(4):
                                ct = half * 4 + j
                                nc.tensor.transpose(
                                    trp[:, j * P:(j + 1) * P],
                                    raw[:, ct * P:(ct + 1) * P], ident)
                            drain(vxt[:, half * 4:(half + 1) * 4,
                                      mc * P:(mc + 1) * P], trp)

                def v_project(mb):
                    vxt = vxts[mb]
                    for cc8 in range(8):
                        pv = vpp.tile([P, 512], f32, tag="vpr")
                        for ct in range(8):
                            nc.tensor.matmul(
                                pv, wv_sb[:, ct, cc8 * P:(cc8 + 1) * P],
                                vxt[:, ct, :],
                                start=(ct == 0), stop=(ct == 7))
                        vst = vstp.tile([P, 512], f32r, tag="vst")
                        drain(vst, pv)
                        nc.sync.dma_start(
                            vp_d.ap()[:, cc8, mb * 512:(mb + 1) * 512], vst)

                v_transpose(0)
                for cc in range(8):
                    nc.sync.dma_start(wv_sb[:, cc, :], wv_d.ap()[:, cc, :])
                v_project(0)
                for mb in range(1, 4):
                    v_transpose(mb)
                    v_project(mb)
            wv_pool.release()

            s_pool = tc.alloc_tile_pool(name="sg", bufs=1)
            s_g = [s_pool.tile([P, 16, 512], f32r, name=f"s{g}")
                   for g in range(2)]

            # ---- Q: project + spectra into s_g
            fep_q, feq, foq = proj_tensor("q", qin, wq_d)
            with tc.tile_pool(name="ftiq", bufs=3) as ftp, \
                 tc.tile_pool(name="fpsq", bufs=3, space="PSUM") as fps:
                for g in range(2):
                    for t in range(8):
                        fti = ftp.tile([P, 8, 2 * P], f32r, tag="fti",
                                       name=f"ftiq_{g}_{t}")
                        nc.sync.dma_start(fti, f_d.ap()[t])
                        rhs_q = (feq if t < 4 else foq)[g]
                        pq = fps.tile([P, 2, 512], f32, tag="pq",
                                      name=f"pq_{g}_{t}")
                        for mt in range(8):
                            st, sp = (mt == 0), (mt == 7)
                            nc.tensor.matmul(pq[:, 0, :], fti[:, mt, 0:P],
                                             rhs_q[:, mt, :], start=st, stop=sp)
                            nc.tensor.matmul(pq[:, 1, :], fti[:, mt, P:2 * P],
                                             rhs_q[:, mt, :], start=st, stop=sp)
                        drain(s_g[g][:, t, :], pq[:, 0, :])
                        drain(s_g[g][:, 8 + t, :], pq[:, 1, :])
            if taps:
                for g in range(2):
                    nc.sync.dma_start(taps["feq"].ap()[g], feq[g])
                    nc.sync.dma_start(taps["foq"].ap()[g], foq[g])
                    nc.sync.dma_start(taps["sq"].ap()[g], s_g[g])
            fep_q.release()

            # ---- K: project + spectra; product overwrites s_g in place
            fep_k, fek, fok = proj_tensor("k", kin, wk_d)
            with tc.tile_pool(name="ftik", bufs=3) as ftp, \
                 tc.tile_pool(name="sppk", bufs=2) as spp, \
                 tc.tile_pool(name="fpsk", bufs=2, space="PSUM") as fps:
                for g in range(2):
                    for t in range(8):
                        fti = ftp.tile([P, 8, 2 * P], f32r, tag="fti",
                                       name=f"ftik_{g}_{t}")
                        nc.sync.dma_start(fti, f_d.ap()[t])
                        rhs_k = (fek if t < 4 else fok)[g]
                        pk = fps.tile([P, 2, 512], f32, tag="pk",
                                      name=f"pk_{g}_{t}")
                        for mt in range(8):
                            st, sp = (mt == 0), (mt == 7)
                            nc.tensor.matmul(pk[:, 0, :], fti[:, mt, 0:P],
                                             rhs_k[:, mt, :], start=st, stop=sp)
                            nc.tensor.matmul(pk[:, 1, :], fti[:, mt, P:2 * P],
                                             rhs_k[:, mt, :], start=st, stop=sp)
                        # spectral product: s_re = qre*kre + qim*kim,
                        # s_im = qim*kre - qre*kim (in-place over q spectra)
                        qre = s_g[g][:, t, :]
                        qim = s_g[g][:, 8 + t, :]
                        kre = pk[:, 0, :]
                        kim = pk[:, 1, :]
                        tm1 = spp.tile([P, 512], f32, tag="tm1")
                        tm2 = spp.tile([P, 512], f32, tag="tm2")
                        tm3 = spp.tile([P, 512], f32, tag="tm3")
                        tm4 = spp.tile([P, 512], f32, tag="tm4")
                        if t == 0:
                            # row 0 of tile 0 = DC (qre*kre); row 0 of tile 8
                            # = nyquist (qim*kim); compute before overwrite
                            dcny = spp.tile([1, 1024], f32, tag="dc",
                                            name=f"dcny_{g}")
                            nc.vector.tensor_tensor(
                                dcny[0:1, 0:512], qre[0:1, :], kre[0:1, :],
                                Alu.mult)
                            nc.vector.tensor_tensor(
                                dcny[0:1, 512:1024], qim[0:1, :], kim[0:1, :],
                                Alu.mult)
                        nc.vector.tensor_tensor(tm1, kre, qre, Alu.mult)
                        nc.vector.tensor_tensor(tm2, kim, qim, Alu.mult)
                        nc.vector.tensor_tensor(tm3, kre, qim, Alu.mult)
                        nc.vector.tensor_tensor(tm4, kim, qre, Alu.mult)
                        ceng = nc.vector if t % 2 == 0 else nc.gpsimd
                        ceng.tensor_tensor(qre, tm1, tm2, Alu.add)
                        ceng.tensor_tensor(qim, tm3, tm4, Alu.subtract)
                        if t == 0:
                            nc.vector.tensor_copy(s_g[g][0:1, 0, :],
                                                  dcny[0:1, 0:512])
                            nc.vector.tensor_copy(s_g[g][0:1, 8, :],
                                                  dcny[0:1, 512:1024])
            if taps:
                for g in range(2):
                    nc.sync.dma_start(taps["sp"].ap()[g], s_g[g])
            fep_k.release()

            def topk_chunk(g, c):
                if True:
                    gt_idx = g * 4 + c
                    topv = smf[:, gt_idx, 0:8]
                    expw = smf[:, gt_idx, 8:16]
                    shmf = smf[:, gt_idx, 16:24]
                    nv0 = smf[:, gt_idx, 24:25]
                    s7 = smf[:, gt_idx, 25:26]
                    r7 = smf[:, gt_idx, 26:27]
                    topi = smu[:, gt_idx, 0:8]
                    shmi = smu[:, gt_idx, 8:16]
                    nc.vector.max(out=topv, in_=corr[g][:, c, :])
                    nc.vector.max_index(out=topi, in_max=topv,
                                        in_values=corr[g][:, c, :])
                    nc.vector.tensor_scalar(nv0, topv[:, 0:1], -1.0, None,
                                            op0=Alu.mult)
                    nc.scalar.activation(
                        expw[:, 0:KTOP], topv[:, 0:KTOP],
                        mybir.ActivationFunctionType.Exp,
                        bias=nv0, scale=1.0)
                    nc.vector.reduce_sum(s7, expw[:, 0:KTOP],
                                         axis=mybir.AxisListType.X)
                    nc.vector.reciprocal(r7, s7)
                    nc.vector.tensor_scalar(expw[:, 0:KTOP], expw[:, 0:KTOP],
                                            r7, None, op0=Alu.mult)
                    nc.vector.tensor_scalar(shmi, topi, 63, None,
                                            op0=Alu.bitwise_and)
                    nc.vector.tensor_copy(shmf, shmi)
                    tmp64 = tmp64a if (c % 2 == 0) else tmp64b
                    for i in range(KTOP):
                        dst = m_all[:, gt_idx, :] if i == 0 else tmp64
                        nc.vector.tensor_scalar(
                            dst, t64f, shmf[:, i:i + 1], expw[:, i:i + 1],
                            op0=Alu.is_equal, op1=Alu.mult)
                        if i > 0:
                            nc.vector.tensor_tensor(
                                m_all[:, gt_idx, :], m_all[:, gt_idx, :],
                                tmp64, Alu.add)

            # ------------- phase G + top-k(0) interleaved
            corr_pool = tc.alloc_tile_pool(name="corr", bufs=1)
            corr = [corr_pool.tile([P, 4, L], f32, name=f"corr{g}")
                    for g in range(2)]

            ODD = (4, 5, 6, 7, 12, 13, 14, 15)
            EVEN = (0, 1, 2, 3, 8, 9, 10, 11)
            with tc.tile_pool(name="gt", bufs=4) as gtp, \
                 tc.tile_pool(name="pbs", bufs=1) as pbsp, \
                 tc.tile_pool(name="gps", bufs=1, space="PSUM") as gps:
                def g_quadrant(g, nn):
                    pB = [gps.tile([P, 512], f32, tag=f"B{c}",
                                   name=f"pB_{g}_{nn}_{c}")
                          for c in range(4)]
                    for kt in ODD:
                        gt = gtp.tile([P, 512], f32r, tag="gt",
                                      name=f"gt_{g}_{nn}_{kt}")
                        nc.sync.dma_start(gt, g_d.ap()[kt, nn])
                        for c in range(4):
                            nc.tensor.matmul(
                                pB[c], s_g[g][:, kt, c * P:(c + 1) * P],
                                gt, start=(kt == 4), stop=(kt == 15))
                    pBs = [pbsp.tile([P, 512], f32, tag=f"Bs{c}",
                                     name=f"pBs_{g}_{nn}_{c}")
                           for c in range(4)]
                    for c in range(4):
                        nc.scalar.copy(pBs[c], pB[c])
                    pA = [gps.tile([P, 512], f32, tag=f"A{c}",
                                   name=f"pA_{g}_{nn}_{c}")
                          for c in range(4)]
                    for kt in EVEN:
                        gt = gtp.tile([P, 512], f32r, tag="gt",
                                      name=f"gt_{g}_{nn}_{kt}")
                        nc.sync.dma_start(gt, g_d.ap()[kt, nn])
                        for c in range(4):
                            nc.tensor.matmul(
                                pA[c], s_g[g][:, kt, c * P:(c + 1) * P],
                                gt, start=(kt == 0), stop=(kt == 11))
                    for c in range(4):
                        nc.vector.tensor_tensor(
                            corr[g][:, c, nn * 512:(nn + 1) * 512],
                            pA[c], pBs[c], Alu.add)
                        nc.vector.tensor_tensor(
                            corr[g][:, c, 1024 + nn * 512:
                                    1024 + (nn + 1) * 512],
                            pA[c], pBs[c], Alu.subtract)

                g_quadrant(0, 0)
                g_quadrant(0, 1)
                g_quadrant(1, 0)
                topk_chunk(0, 0)
                topk_chunk(0, 1)
                g_quadrant(1, 1)
                topk_chunk(0, 2)
                topk_chunk(0, 3)

            if taps:
                for g in range(2):
                    nc.sync.dma_start(taps["corr"].ap()[g], corr[g])
            for c in range(4):
                topk_chunk(1, c)
            if taps:
                nc.sync.dma_start(taps["m"].ap(), m_all)
            corr_pool.release()
            s_pool.release()

            # ------------- roll + scramble + final projection
            wo_pool = tc.alloc_tile_pool(name="wo", bufs=1)
            wo_sb = wo_pool.tile([P, 8, D], f32r)
            for cc in range(8):
                nc.sync.dma_start(wo_sb[:, cc, :], wo_d.ap()[:, cc, :])
            r2t_pool = tc.alloc_tile_pool(name="r2t", bufs=1)
            r2t = r2t_pool.tile([P, 8, 64, 32], f32r)   # (p, kc, d, hp)

            with tc.tile_pool(name="vsin", bufs=2) as vsinp, \
                 tc.tile_pool(name="mtp", bufs=2, space="PSUM") as mtp, \
                 tc.tile_pool(name="rollp", bufs=3, space="PSUM") as rop:
                for h2 in range(8):
                    vsin = vsinp.tile([P, L], f32r, tag="vsin",
                                      name=f"vsin_{h2}")
                    nc.sync.dma_start(vsin, vp_d.ap()[:, h2, :])
                    for hh in range(2):
                        h = 2 * h2 + hh
                        pb = (h % 2) * 64
                        pm = mtp.tile([64, 64], f32, tag="mt")
                        nc.tensor.transpose(
                            pm, m_all[pb:pb + 64, h // 2, :],
                            ident2[pb:pb + 64, :])
                        nc.scalar.copy(mp_all[pb:pb + 64, h // 2, :], pm)
                        pr = rop.tile([P, 8, 2, 64], f32, tag="roll",
                                      name=f"pr_{h}")
                        for kc in range(8):
                            for pp_ in range(2):
                                lc = kc + 8 * pp_
                                nc.tensor.matmul(
                                    pr[:, kc, pp_, :],
                                    vsin[pb:pb + 64, lc * P:(lc + 1) * P],
                                    mp_all[pb:pb + 64, h // 2, :],
                                    start=True, stop=True)
                        # one wide rearranged drain per head; heads 0-7 on
                        # scalar (DVE still on top-k), rest alternate
                        if h < 8 or h % 2 == 1:
                            nc.scalar.copy(
                                r2t[:, :, :, h * 2:h * 2 + 2],
                                pr.rearrange("p kc j d -> p kc d j"))
                        else:
                            nc.vector.tensor_copy(
                                r2t[:, :, :, h * 2:h * 2 + 2],
                                pr.rearrange("p kc j d -> p kc d j"))
            if taps:
                nc.sync.dma_start(taps["r2t"].ap(), r2t)
            with tc.tile_pool(name="fpp", bufs=4, space="PSUM") as fpp, \
                 tc.tile_pool(name="osb", bufs=2) as osbp:
                for a in range(16):
                    for j2 in range(2):
                        pf = fpp.tile([P, 512], f32, tag="fin")
                        for kc in range(8):
                            nc.tensor.matmul(
                                pf, r2t[:, kc, 4 * a:4 * a + 4, :],
                                wo_sb[:, kc, j2 * 512:(j2 + 1) * 512],
                                start=(kc == 0), stop=(kc == 7))
                        osb = osbp.tile([P, 512], f32, tag="osb")
                        drain(osb, pf)
                        nc.sync.dma_start(
                            out_d.ap()[a * P:(a + 1) * P,
                                       j2 * 512:(j2 + 1) * 512], osb)
            r2t_pool.release()
            wo_pool.release()

    _split_excess_waits(nc, mybir)
    return nc


def _get_program():
    if "nc" not in _prog_cache:
        _prog_cache["nc"] = _build_program()
    return _prog_cache["nc"]


# ---------------------------------------------------------------- entry point
def _last_in_maps_get():
    return _prog_cache["last_in_maps"]


def kernel(queries, keys, values, wq, wk, wv, wo):
    from concourse.bass_utils import run_bass_kernel_spmd

    queries = np.ascontiguousarray(queries, np.float32)
    keys = np.ascontiguousarray(keys, np.float32)
    values = np.ascontiguousarray(values, np.float32)

    if "fg" not in _prog_cache:
        _prog_cache["fg"] = _host_constants()
    fmat, gmat = _prog_cache["fg"]
    consts = {
        "fmat": fmat, "gmat": gmat,
        "wq_t": _round_fp32r(_tile_w(np.asarray(wq, np.float32))),
        "wk_t": _round_fp32r(_tile_w(np.asarray(wk, np.float32))),
        "wv_t": _round_fp32r(_tile_w(np.asarray(wv, np.float32))),
        "wo_t": _round_fp32r(_tile_w(np.asarray(wo, np.float32))),
    }

    nc = _get_program()
    in_maps = []
    for b in range(NCORES):
        in_maps.append({
            "qin": np.ascontiguousarray(queries[b]),
            "kin": np.ascontiguousarray(keys[b]),
            "vin": np.ascontiguousarray(values[b]),
            **consts,
        })
    _prog_cache["last_in_maps"] = in_maps
    res = run_bass_kernel_spmd(nc, in_maps, core_ids=list(range(NCORES)),
                               trace=False)
    out = np.stack([res.results[b]["out"] for b in range(NCORES)], axis=0)
    return out.astype(np.float32)


# revision 38
# speedup vs baseline: 1.0375x; 1.0375x over previous
"""Trainium2 Bass kernel for nn_AutoCorrelation (Autoformer AutoCorrelation).

Math identical to the validated baseline (dense rfft-as-matmul with radix-2
time fold, DVE top-8, mod-64 roll as per-head 64x64 matmuls, scrambling
reshape folded into the final projection's lhsT layout). All matmul data
stays float32r (bf16 anywhere in the Q/K correlation path measurably breaks
the top-k selection: numpy study gives rel err 1.2e-2..3.4e-2 vs 5e-4).

Restructured for PE throughput vs the first working version (1.10 ms -> ~0.66 ms
neuron-profile device exec):
  - q/k loaded + transposed ONCE (the old version re-did both per channel
    group): the radix-2 time fold now happens on the RAW inputs (DVE adds,
    fold commutes with the projection), so folded spectra inputs come
    straight out of the projection and the PSUM fold drain disappears.
  - Q spectra land in SBUF; K spectra stay in PSUM and the spectral product
    overwrites the Q spectra in place (saves 32 PSUM->SBUF copies and all
    pool-lifetime conflicts nest LIFO as the tile framework requires).
  - The V path runs FIRST (it is independent): it warms the PE before the
    projections and parks projected V^T in DRAM, re-read at the roll phase,
    which frees 64KB/partition through the F/G/top-k phases.
  - PSUM drains alternate DVE/scalar; G-combine partner terms for odd t go
    to gpsimd; top-k(0) chunks are emitted between G(g1) quadrants so the
    DVE queue never blocks the G combines the PE waits on.
  - G accumulates the odd-k tiles first so their PSUM banks drain to SBUF
    while the even-k tiles accumulate, keeping all 8 banks cycling.

Sharding: data-parallel over batch B=8 -> one batch element per NeuronCore.
"""
import numpy as np

B, L, D, H, Dh = 8, 2048, 1024, 16, 64
KTOP = 7
NCORES = 8
P = 128

_prog_cache = {}


# ---------------------------------------------------------------- host helpers
def _round_fp32r(x):
    """Round fp32 to the ~13-bit-mantissa fp32r grid (matches measured HW)."""
    b = np.ascontiguousarray(x, dtype=np.float32).view(np.uint32)
    half = np.uint32(1 << 9)
    keep = np.uint32(0xFFFFFFFF) ^ np.uint32((1 << 10) - 1)
    out = ((b + half) & keep).view(np.float32)
    return np.where(np.isfinite(out), out, 0.0).astype(np.float32)


def _k_of(j, r):
    """k-index of packed row/col 128*j+r in the parity-permuted layout."""
    if j < 4:
        return 2 * (128 * j + r), "re"
    if j < 8:
        return 2 * (128 * (j - 4) + r) + 1, "re"
    if j < 12:
        return 2 * (128 * (j - 8) + r), "im"
    return 2 * (128 * (j - 12) + r) + 1, "im"


def _build_FG():
    """Radix-2-folded DFT matrices in the parity-permuted packed layout.

    Fh (1024, 2048): cols for even-k tiles multiply Qe = q[:1024]+q[1024:],
    odd-k tiles multiply Qo = q[:1024]-q[1024:]; sqrt(c_k/L) folded in.
    G (2048, 1024): rows give n in [0,1024); corr[n+1024] is recovered from
    the even-tile/odd-tile partial sums as A - B. Tile 8 row 0 carries the
    nyquist term (its natural occupant, im k=0, is identically zero)."""
    m = np.arange(1024)[:, None].astype(np.float64)
    n = np.arange(1024)[None, :].astype(np.float64)
    Fh = np.zeros((1024, 2048), dtype=np.float64)
    G = np.zeros((2048, 1024), dtype=np.float64)
    for j in range(16):
        for r in range(128):
            k, ri = _k_of(j, r)
            col = 128 * j + r
            if j == 8 and r == 0:  # nyquist slot
                Fh[:, col] = (np.cos(np.pi * m[:, 0]) * np.sqrt(1.0 / L))
                G[col, :] = np.cos(np.pi * n[0])
                continue
            ck = 1.0 if k == 0 else 2.0
            if ri == "re":
                Fh[:, col] = np.cos(2 * np.pi * m[:, 0] * k / L) * np.sqrt(ck / L)
                G[col, :] = np.cos(2 * np.pi * k * n[0] / L)
            else:
                Fh[:, col] = -np.sin(2 * np.pi * m[:, 0] * k / L) * np.sqrt(ck / L)
                G[col, :] = -np.sin(2 * np.pi * k * n[0] / L)
    return Fh.astype(np.float32), G.astype(np.float32)


def _tile_w(w):
    """(1024, 1024) -> (128, 8, 1024): [p, o, c] = w[o*128+p, c]."""
    return np.ascontiguousarray(
        w.reshape(8, P, D).transpose(1, 0, 2)).astype(np.float32)


def _host_constants():
    Fh, G = _build_FG()
    # fmat[t, p, mt] = [Fh[mt*P+p, t-block] | Fh[mt*P+p, (8+t)-block]]
    # (p-major so the per-t DMA into a [128, 8, 256] tile is contiguous)
    fmat = np.zeros((8, P, 8, 2 * P), dtype=np.float32)
    for t in range(8):
        for mt in range(8):
            fmat[t, :, mt, 0:P] = Fh[mt * P:(mt + 1) * P, t * P:(t + 1) * P]
            fmat[t, :, mt, P:2 * P] = Fh[mt * P:(mt + 1) * P,
                                         (8 + t) * P:(9 + t) * P]
    # gmat[kt, nn] = G[kt*128:(kt+1)*128, nn*512:(nn+1)*512], nn in 0..1
    gmat = np.zeros((16, 2, P, 512), dtype=np.float32)
    for kt in range(16):
        for nn in range(2):
            gmat[kt, nn] = G[kt * P:(kt + 1) * P, nn * 512:(nn + 1) * 512]
    return _round_fp32r(fmat), _round_fp32r(gmat)


# ---------------------------------------------------------------- wait splitting
def _split_excess_waits(nc, mybir):
    """walrus here allows max ONE sem wait per instruction; hoist extras onto
    injected NoOps on the same engine."""
    ctr = 0
    for f in nc.m.functions:
        for bblk in f.blocks:
            insts = bblk.instructions
            i = 0
            while i < len(insts):
                ins = insts[i]
                si = ins.sync_info
                if si is not None and len(si.on_wait) > 1:
                    waits = list(si.on_wait)
                    for w in waits[:-1]:
                        nop = mybir.InstNoOp(name=f"WSPL-{ctr}", ins=[], outs=[])
                        ctr += 1
                        nop.engine = ins.engine
                        nop.sync_info = mybir.SyncInfo(on_wait=[w], on_update=[])
                        insts.insert(i, nop)
                        i += 1
                    ins.sync_info = mybir.SyncInfo(
                        on_wait=[waits[-1]], on_update=list(si.on_update))
                i += 1


# ---------------------------------------------------------------- bass program
def _build_program():
    import concourse.bass as bass
    import concourse.mybir as mybir
    from concourse.tile import TileContext
    from concourse.masks import make_identity

    f32 = mybir.dt.float32
    f32r = mybir.dt.float32r
    i32 = mybir.dt.int32
    u32 = mybir.dt.uint32
    Alu = mybir.AluOpType

    nc = bass.Bass("TRN2", target_bir_lowering=False)

    qin = nc.dram_tensor("qin", (L, D), f32, kind="ExternalInput")
    kin = nc.dram_tensor("kin", (L, D), f32, kind="ExternalInput")
    vin = nc.dram_tensor("vin", (L, D), f32, kind="ExternalInput")
    wq_d = nc.dram_tensor("wq_t", (P, 8, D), f32r, kind="ExternalInput")
    wk_d = nc.dram_tensor("wk_t", (P, 8, D), f32r, kind="ExternalInput")
    wv_d = nc.dram_tensor("wv_t", (P, 8, D), f32r, kind="ExternalInput")
    wo_d = nc.dram_tensor("wo_t", (P, 8, D), f32r, kind="ExternalInput")
    f_d = nc.dram_tensor("fmat", (8, P, 8, 2 * P), f32r, kind="ExternalInput")
    g_d = nc.dram_tensor("gmat", (16, 2, P, 512), f32r, kind="ExternalInput")
    out_d = nc.dram_tensor("out", (L, D), f32, kind="ExternalOutput")
    vp_d = nc.dram_tensor("vproj_dram", (P, 8, L), f32r, kind="Internal")
    taps = {}
    if _prog_cache.get("debug_taps"):
        taps["feq"] = nc.dram_tensor("tap_feq", (2, P, 8, 512), f32r,
                                     kind="ExternalOutput")
        taps["foq"] = nc.dram_tensor("tap_foq", (2, P, 8, 512), f32r,
                                     kind="ExternalOutput")
        taps["sq"] = nc.dram_tensor("tap_sq", (2, P, 16, 512), f32r,
                                    kind="ExternalOutput")
        taps["sp"] = nc.dram_tensor("tap_sp", (2, P, 16, 512), f32r,
                                    kind="ExternalOutput")
        taps["corr"] = nc.dram_tensor("tap_corr", (2, P, 4, L), f32,
                                      kind="ExternalOutput")
        taps["m"] = nc.dram_tensor("tap_m", (P, 8, 64), f32,
                                   kind="ExternalOutput")
        taps["vp"] = nc.dram_tensor("tap_vp", (P, 8, L), f32,
                                    kind="ExternalOutput")
        taps["r2t"] = nc.dram_tensor("tap_r2t", (P, 8, 64, 32), f32r,
                                     kind="ExternalOutput")

    _rr = [0]

    def drain(dst, src_ap):
        # PSUM -> SBUF copy. f32r destinations must go through the DVE
        # (scalar/activation f32r output is not trustworthy); plain f32
        # copies alternate DVE/scalar to spread load.
        if _rr[0] % 2 == 0:
            nc.vector.tensor_copy(dst, src_ap)
        else:
            nc.scalar.copy(dst, src_ap)
        _rr[0] += 1

    with TileContext(nc) as tc:
        with tc.tile_pool(name="const", bufs=1) as cp:
            ident = cp.tile([P, P], f32)
            make_identity(nc, ident)
            # (128, 64) with 1 at (p, p % 64): a 64x64 identity at base 0 or 64
            ident2 = cp.tile([P, 64], f32)
            nc.gpsimd.memset(ident2, 0.0)
            nc.gpsimd.affine_select(
                out=ident2, in_=ident2, compare_op=Alu.not_equal, fill=1.0,
                base=0, channel_multiplier=1, pattern=[[-1, 64]])
            nc.gpsimd.affine_select(
                out=ident2, in_=ident2, compare_op=Alu.not_equal, fill=1.0,
                base=-64, channel_multiplier=1, pattern=[[-1, 64]])
            # T64f[p, s] = (p - s) & 63 as fp32
            t64i = cp.tile([P, 64], i32)
            nc.gpsimd.iota(t64i, pattern=[[-1, 64]], base=0, channel_multiplier=1)
            nc.vector.tensor_scalar(t64i, t64i, 63, None, op0=Alu.bitwise_and)
            t64f = cp.tile([P, 64], f32)
            nc.vector.tensor_copy(t64f, t64i)
            m_all = cp.tile([P, 8, 64], f32)    # roll matrices, (c, s)
            mp_all = cp.tile([P, 8, 64], f32r)  # transposed,  (s, t) per head
            smf = cp.tile([P, 8, 32], f32)      # per-cc top-k scratch
            smu = cp.tile([P, 8, 16], u32)
            tmp64a = cp.tile([P, 64], f32)
            tmp64b = cp.tile([P, 64], f32)

            # ------------- phase P/F: per tensor: fold -> transpose ->
            # project -> spectra.  Q spectra land in s_g; K spectra stay in
            # PSUM and the spectral product overwrites s_g in place.
            def proj_tensor(name, src_d, w_d):
                fep = tc.alloc_tile_pool(name=f"fe{name}", bufs=1)
                fe = [fep.tile([P, 8, 512], f32r, name=f"fe{name}{g}")
                      for g in range(2)]
                fo = [fep.tile([P, 8, 512], f32r, name=f"fo{name}{g}")
                      for g in range(2)]
                wpool = tc.alloc_tile_pool(name=f"w{name}", bufs=1)
                w_sb = wpool.tile([P, 8, D], f32r, name=f"w_{name}")
                for cc in range(8):
                    nc.sync.dma_start(w_sb[:, cc, :], w_d.ap()[:, cc, :])
                with tc.tile_pool(name=f"raw{name}", bufs=2) as rawp, \
                     tc.tile_pool(name=f"eo{name}", bufs=2) as eop, \
                     tc.tile_pool(name=f"xt{name}", bufs=3) as xtp, \
                     tc.tile_pool(name=f"trp{name}", bufs=2,
                                  space="PSUM") as trpp, \
                     tc.tile_pool(name=f"pp{name}", bufs=3,
                                  space="PSUM") as prjp:
                    for mc in range(8):
                        rlo = rawp.tile([P, D], f32, tag="rlo")
                        rhi = rawp.tile([P, D], f32, tag="rhi")
                        nc.sync.dma_start(rlo, src_d.ap()[mc * P:(mc + 1) * P, :])
                        nc.sync.dma_start(
                            rhi, src_d.ap()[(mc + 8) * P:(mc + 9) * P, :])
                        for br, op in ((0, Alu.add), (1, Alu.subtract)):
                            eo = eop.tile([P, D], f32, tag="eo",
                                          name=f"eo_{name}{mc}{br}")
                            nc.vector.tensor_tensor(eo, rlo, rhi, op)
                            # transpose eo -> xt (c-part, m) then project
                            xt = xtp.tile([P, 8, P], f32r, tag="xt",
                                          name=f"xt_{name}{mc}{br}")
                            for half in range(2):
                                trp = trpp.tile([P, 512], f32, tag="tr")
                                for j in range(4):
                                    ct = half * 4 + j
                                    nc.tensor.transpose(
                                        trp[:, j * P:(j + 1) * P],
                                        eo[:, ct * P:(ct + 1) * P], ident)
                                drain(xt[:, half * 4:(half + 1) * 4, :], trp)
                            dst = fe if br == 0 else fo
                            pr = prjp.tile([P, 2, 512], f32, tag="pr",
                                           name=f"pr_{name}{mc}{br}")
                            for g in range(2):
                                for cc in range(8):
                                    nc.tensor.matmul(
                                        pr[:, g, :], xt[:, cc, :],
                                        w_sb[:, cc, g * 512:(g + 1) * 512],
                                        start=(cc == 0), stop=(cc == 7))
                                drain(dst[g][:, mc, :], pr[:, g, :])
                wpool.release()
                return fep, fe, fo

            # ------------- phase V first (independent; warms the PE);
            # projected V^T goes to DRAM and is re-read at the roll phase
            wv_pool = tc.alloc_tile_pool(name="wv", bufs=1)
            wv_sb = wv_pool.tile([P, 8, D], f32r)
            with tc.tile_pool(name="vraw", bufs=2) as vrawp, \
                 tc.tile_pool(name="vxt", bufs=2) as vxtp, \
                 tc.tile_pool(name="vst", bufs=4) as vstp, \
                 tc.tile_pool(name="vtrp", bufs=2, space="PSUM") as vtrpp, \
                 tc.tile_pool(name="vpp", bufs=4, space="PSUM") as vpp:
                vxts = [None] * 4

                def v_transpose(mb):
                    vxt = vxtp.tile([P, 8, 512], f32r, tag="vxt",
                                    name=f"vxt_{mb}")
                    vxts[mb] = vxt
                    for mc in range(4):
                        raw = vrawp.tile([P, D], f32, tag="vraw")
                        nc.sync.dma_start(
                            raw, vin.ap()[(mb * 4 + mc) * P:
                                          (mb * 4 + mc + 1) * P, :])
                        for half in range(2):
                            trp = vtrpp.tile([P, 512], f32, tag="vtr")
                            for j in range(4):
                                ct = half * 4 + j
                                nc.tensor.transpose(
                                    trp[:, j * P:(j + 1) * P],
                                    raw[:, ct * P:(ct + 1) * P], ident)
                            drain(vxt[:, half * 4:(half + 1) * 4,
                                      mc * P:(mc + 1) * P], trp)

                def v_project(mb):
                    vxt = vxts[mb]
                    for cc8 in range(8):
                        pv = vpp.tile([P, 512], f32, tag="vpr")
                        for ct in range(8):
                            nc.tensor.matmul(
                                pv, wv_sb[:, ct, cc8 * P:(cc8 + 1) * P],
                                vxt[:, ct, :],
                                start=(ct == 0), stop=(ct == 7))
                        vst = vstp.tile([P, 512], f32r, tag="vst")
                        drain(vst, pv)
                        nc.sync.dma_start(
                            vp_d.ap()[:, cc8, mb * 512:(mb + 1) * 512], vst)

                v_transpose(0)
                for cc in range(8):
                    nc.sync.dma_start(wv_sb[:, cc, :], wv_d.ap()[:, cc, :])
                v_project(0)
                for mb in range(1, 4):
                    v_transpose(mb)
                    v_project(mb)
            wv_pool.release()

            s_pool = tc.alloc_tile_pool(name="sg", bufs=1)
            s_g = [s_pool.tile([P, 16, 512], f32r, name=f"s{g}")
                   for g in range(2)]

            # ---- Q: project + spectra into s_g
            fep_q, feq, foq = proj_tensor("q", qin, wq_d)
            with tc.tile_pool(name="ftiq", bufs=3) as ftp, \
                 tc.tile_pool(name="fpsq", bufs=3, space="PSUM") as fps:
                for g in range(2):
                    for t in range(8):
                        fti = ftp.tile([P, 8, 2 * P], f32r, tag="fti",
                                       name=f"ftiq_{g}_{t}")
                        nc.sync.dma_start(fti, f_d.ap()[t])
                        rhs_q = (feq if t < 4 else foq)[g]
                        pq = fps.tile([P, 2, 512], f32, tag="pq",
                                      name=f"pq_{g}_{t}")
                        for mt in range(8):
                            st, sp = (mt == 0), (mt == 7)
                            nc.tensor.matmul(pq[:, 0, :], fti[:, mt, 0:P],
                                             rhs_q[:, mt, :], start=st, stop=sp)
                            nc.tensor.matmul(pq[:, 1, :], fti[:, mt, P:2 * P],
                                             rhs_q[:, mt, :], start=st, stop=sp)
                        drain(s_g[g][:, t, :], pq[:, 0, :])
                        drain(s_g[g][:, 8 + t, :], pq[:, 1, :])
            if taps:
                for g in range(2):
                    nc.sync.dma_start(taps["feq"].ap()[g], feq[g])
                    nc.sync.dma_start(taps["foq"].ap()[g], foq[g])
                    nc.sync.dma_start(taps["sq"].ap()[g], s_g[g])
            fep_q.release()

            # ---- K: project + spectra; product overwrites s_g in place
            fep_k, fek, fok = proj_tensor("k", kin, wk_d)
            with tc.tile_pool(name="ftik", bufs=3) as ftp, \
                 tc.tile_pool(name="sppk", bufs=2) as spp, \
                 tc.tile_pool(name="fpsk", bufs=2, space="PSUM") as fps:
                for g in range(2):
                    for t in range(8):
                        fti = ftp.tile([P, 8, 2 * P], f32r, tag="fti",
                                       name=f"ftik_{g}_{t}")
                        nc.sync.dma_start(fti, f_d.ap()[t])
                        rhs_k = (fek if t < 4 else fok)[g]
                        pk = fps.tile([P, 2, 512], f32, tag="pk",
                                      name=f"pk_{g}_{t}")
                        for mt in range(8):
                            st, sp = (mt == 0), (mt == 7)
                            nc.tensor.matmul(pk[:, 0, :], fti[:, mt, 0:P],
                                             rhs_k[:, mt, :], start=st, stop=sp)
                            nc.tensor.matmul(pk[:, 1, :], fti[:, mt, P:2 * P],
                                             rhs_k[:, mt, :], start=st, stop=sp)
                        # spectral product: s_re = qre*kre + qim*kim,
                        # s_im = qim*kre - qre*kim (in-place over q spectra)
                        qre = s_g[g][:, t, :]
                        qim = s_g[g][:, 8 + t, :]
                        kre = pk[:, 0, :]
                        kim = pk[:, 1, :]
                        tm1 = spp.tile([P, 512], f32, tag="tm1")
                        tm2 = spp.tile([P, 512], f32, tag="tm2")
                        tm3 = spp.tile([P, 512], f32, tag="tm3")
                        tm4 = spp.tile([P, 512], f32, tag="tm4")
                        if t == 0:
                            # row 0 of tile 0 = DC (qre*kre); row 0 of tile 8
                            # = nyquist (qim*kim); compute before overwrite
                            dcny = spp.tile([1, 1024], f32, tag="dc",
                                            name=f"dcny_{g}")
                            nc.vector.tensor_tensor(
                                dcny[0:1, 0:512], qre[0:1, :], kre[0:1, :],
                                Alu.mult)
                            nc.vector.tensor_tensor(
                                dcny[0:1, 512:1024], qim[0:1, :], kim[0:1, :],
                                Alu.mult)
                        nc.vector.tensor_tensor(tm1, kre, qre, Alu.mult)
                        nc.vector.tensor_tensor(tm2, kim, qim, Alu.mult)
                        nc.vector.tensor_tensor(tm3, kre, qim, Alu.mult)
                        nc.vector.tensor_tensor(tm4, kim, qre, Alu.mult)
                        ceng = nc.vector if t % 2 == 0 else nc.gpsimd
                        ceng.tensor_tensor(qre, tm1, tm2, Alu.add)
                        ceng.tensor_tensor(qim, tm3, tm4, Alu.subtract)
                        if t == 0:
                            nc.vector.tensor_copy(s_g[g][0:1, 0, :],
                                                  dcny[0:1, 0:512])
                            nc.vector.tensor_copy(s_g[g][0:1, 8, :],
                                                  dcny[0:1, 512:1024])
            if taps:
                for g in range(2):
                    nc.sync.dma_start(taps["sp"].ap()[g], s_g[g])
            fep_k.release()

            def topk_chunk(g, c):
                if True:
                    gt_idx = g * 4 + c
                    topv = smf[:, gt_idx, 0:8]
                    expw = smf[:, gt_idx, 8:16]
                    shmf = smf[:, gt_idx, 16:24]
                    nv0 = smf[:, gt_idx, 24:25]
                    s7 = smf[:, gt_idx, 25:26]
                    r7 = smf[:, gt_idx, 26:27]
                    topi = smu[:, gt_idx, 0:8]
                    shmi = smu[:, gt_idx, 8:16]
                    nc.vector.max(out=topv, in_=corr[g][:, c, :])
                    nc.vector.max_index(out=topi, in_max=topv,
                                        in_values=corr[g][:, c, :])
                    nc.vector.tensor_scalar(nv0, topv[:, 0:1], -1.0, None,
                                            op0=Alu.mult)
                    nc.scalar.activation(
                        expw[:, 0:KTOP], topv[:, 0:KTOP],
                        mybir.ActivationFunctionType.Exp,
                        bias=nv0, scale=1.0)
                    nc.vector.reduce_sum(s7, expw[:, 0:KTOP],
                                         axis=mybir.AxisListType.X)
                    nc.vector.reciprocal(r7, s7)
                    nc.vector.tensor_scalar(expw[:, 0:KTOP], expw[:, 0:KTOP],
                                            r7, None, op0=Alu.mult)
                    nc.vector.tensor_scalar(shmi, topi, 63, None,
                                            op0=Alu.bitwise_and)
                    nc.vector.tensor_copy(shmf, shmi)
                    tmp64 = tmp64a if (c % 2 == 0) else tmp64b
                    for i in range(KTOP):
                        dst = m_all[:, gt_idx, :] if i == 0 else tmp64
                        nc.vector.tensor_scalar(
                            dst, t64f, shmf[:, i:i + 1], expw[:, i:i + 1],
                            op0=Alu.is_equal, op1=Alu.mult)
                        if i > 0:
                            nc.vector.tensor_tensor(
                                m_all[:, gt_idx, :], m_all[:, gt_idx, :],
                                tmp64, Alu.add)

            # ------------- phase G + top-k(0) interleaved
            corr_pool = tc.alloc_tile_pool(name="corr", bufs=1)
            corr = [corr_pool.tile([P, 4, L], f32, name=f"corr{g}")
                    for g in range(2)]

            ODD = (4, 5, 6, 7, 12, 13, 14, 15)
            EVEN = (0, 1, 2, 3, 8, 9, 10, 11)
            with tc.tile_pool(name="gt", bufs=4) as gtp, \
                 tc.tile_pool(name="pbs", bufs=1) as pbsp, \
                 tc.tile_pool(name="gps", bufs=1, space="PSUM") as gps:
                def g_quadrant(g, nn):
                    pB = [gps.tile([P, 512], f32, tag=f"B{c}",
                                   name=f"pB_{g}_{nn}_{c}")
                          for c in range(4)]
                    for kt in ODD:
                        gt = gtp.tile([P, 512], f32r, tag="gt",
                                      name=f"gt_{g}_{nn}_{kt}")
                        nc.sync.dma_start(gt, g_d.ap()[kt, nn])
                        for c in range(4):
                            nc.tensor.matmul(
                                pB[c], s_g[g][:, kt, c * P:(c + 1) * P],
                                gt, start=(kt == 4), stop=(kt == 15))
                    pBs = [pbsp.tile([P, 512], f32, tag=f"Bs{c}",
                                     name=f"pBs_{g}_{nn}_{c}")
                           for c in range(4)]
                    for c in range(4):
                        nc.scalar.copy(pBs[c], pB[c])
                    pA = [gps.tile([P, 512], f32, tag=f"A{c}",
                                   name=f"pA_{g}_{nn}_{c}")
                          for c in range(4)]
                    for kt in EVEN:
                        gt = gtp.tile([P, 512], f32r, tag="gt",
                                      name=f"gt_{g}_{nn}_{kt}")
                        nc.sync.dma_start(gt, g_d.ap()[kt, nn])
                        for c in range(4):
                            nc.tensor.matmul(
                                pA[c], s_g[g][:, kt, c * P:(c + 1) * P],
                                gt, start=(kt == 0), stop=(kt == 11))
                    for c in range(4):
                        nc.vector.tensor_tensor(
                            corr[g][:, c, nn * 512:(nn + 1) * 512],
                            pA[c], pBs[c], Alu.add)
                        nc.vector.tensor_tensor(
                            corr[g][:, c, 1024 + nn * 512:
                                    1024 + (nn + 1) * 512],
                            pA[c], pBs[c], Alu.subtract)

                g_quadrant(0, 0)
                g_quadrant(0, 1)
                g_quadrant(1, 0)
                topk_chunk(0, 0)
                topk_chunk(0, 1)
                g_quadrant(1, 1)
                topk_chunk(0, 2)
                topk_chunk(0, 3)

            if taps:
                for g in range(2):
                    nc.sync.dma_start(taps["corr"].ap()[g], corr[g])
            for c in range(4):
                topk_chunk(1, c)
            if taps:
                nc.sync.dma_start(taps["m"].ap(), m_all)
            corr_pool.release()
            s_pool.release()

            # ------------- roll + scramble + final projection
            wo_pool = tc.alloc_tile_pool(name="wo", bufs=1)
            wo_sb = wo_pool.tile([P, 8, D], f32r)
            for cc in range(8):
                nc.sync.dma_start(wo_sb[:, cc, :], wo_d.ap()[:, cc, :])
            r2t_pool = tc.alloc_tile_pool(name="r2t", bufs=1)
            r2t = r2t_pool.tile([P, 8, 64, 32], f32r)   # (p, kc, d, hp)

            with tc.tile_pool(name="vsin", bufs=2) as vsinp, \
                 tc.tile_pool(name="mtp", bufs=2, space="PSUM") as mtp, \
                 tc.tile_pool(name="rollp", bufs=6, space="PSUM") as rop:
                for h2 in range(8):
                    vsin = vsinp.tile([P, L], f32r, tag="vsin",
                                      name=f"vsin_{h2}")
                    nc.sync.dma_start(vsin, vp_d.ap()[:, h2, :])
                    for hh in range(2):
                        h = 2 * h2 + hh
                        pb = (h % 2) * 64
                        pm = mtp.tile([64, 64], f32, tag="mt")
                        nc.tensor.transpose(
                            pm, m_all[pb:pb + 64, h // 2, :],
                            ident2[pb:pb + 64, :])
                        nc.scalar.copy(mp_all[pb:pb + 64, h // 2, :], pm)
                        for kc in range(8):
                            pr = rop.tile([P, 2, 64], f32, tag="roll",
                                          name=f"pr_{h}_{kc}")
                            for pp_ in range(2):
                                lc = kc + 8 * pp_
                                nc.tensor.matmul(
                                    pr[:, pp_, :],
                                    vsin[pb:pb + 64, lc * P:(lc + 1) * P],
                                    mp_all[pb:pb + 64, h // 2, :],
                                    start=True, stop=True)
                            # heads 0-7: scalar only (DVE still on top-k);
                            # heads 8-15: alternate DVE/scalar
                            if h < 8 or (h * 8 + kc) % 2 == 1:
                                nc.scalar.copy(
                                    r2t[:, kc, :, h * 2:h * 2 + 2],
                                    pr.rearrange("p j d -> p d j"))
                            else:
                                nc.vector.tensor_copy(
                                    r2t[:, kc, :, h * 2:h * 2 + 2],
                                    pr.rearrange("p j d -> p d j"))
            if taps:
                nc.sync.dma_start(taps["r2t"].ap(), r2t)
            with tc.tile_pool(name="fpp", bufs=4, space="PSUM") as fpp, \
                 tc.tile_pool(name="osb", bufs=2) as osbp:
                for a in range(16):
                    for j2 in range(2):
                        pf = fpp.tile([P, 512], f32, tag="fin")
                        for kc in range(8):
                            nc.tensor.matmul(
                                pf, r2t[:, kc, 4 * a:4 * a + 4, :],
                                wo_sb[:, kc, j2 * 512:(j2 + 1) * 512],
                                start=(kc == 0), stop=(kc == 7))
                        osb = osbp.tile([P, 512], f32, tag="osb")
                        drain(osb, pf)
                        nc.sync.dma_start(
                            out_d.ap()[a * P:(a + 1) * P,
                                       j2 * 512:(j2 + 1) * 512], osb)
            r2t_pool.release()
            wo_pool.release()

    _split_excess_waits(nc, mybir)
    return nc


def _get_program():
    if "nc" not in _prog_cache:
        _prog_cache["nc"] = _build_program()
    return _prog_cache["nc"]


# ---------------------------------------------------------------- entry point
def _last_in_maps_get():
    return _prog_cache["last_in_maps"]


def kernel(queries, keys, values, wq, wk, wv, wo):
    from concourse.bass_utils import run_bass_kernel_spmd

    queries = np.ascontiguousarray(queries, np.float32)
    keys = np.ascontiguousarray(keys, np.float32)
    values = np.ascontiguousarray(values, np.float32)

    if "fg" not in _prog_cache:
        _prog_cache["fg"] = _host_constants()
    fmat, gmat = _prog_cache["fg"]
    consts = {
        "fmat": fmat, "gmat": gmat,
        "wq_t": _round_fp32r(_tile_w(np.asarray(wq, np.float32))),
        "wk_t": _round_fp32r(_tile_w(np.asarray(wk, np.float32))),
        "wv_t": _round_fp32r(_tile_w(np.asarray(wv, np.float32))),
        "wo_t": _round_fp32r(_tile_w(np.asarray(wo, np.float32))),
    }

    nc = _get_program()
    in_maps = []
    for b in range(NCORES):
        in_maps.append({
            "qin": np.ascontiguousarray(queries[b]),
            "kin": np.ascontiguousarray(keys[b]),
            "vin": np.ascontiguousarray(values[b]),
            **consts,
        })
    _prog_cache["last_in_maps"] = in_maps
    res = run_bass_kernel_spmd(nc, in_maps, core_ids=list(range(NCORES)),
                               trace=False)
    out = np.stack([res.results[b]["out"] for b in range(NCORES)], axis=0)
    return out.astype(np.float32)


# revision 40
# speedup vs baseline: 1.0680x; 1.0294x over previous
"""Trainium2 Bass kernel for nn_AutoCorrelation (Autoformer AutoCorrelation).

Math identical to the validated baseline (dense rfft-as-matmul with radix-2
time fold, DVE top-8, mod-64 roll as per-head 64x64 matmuls, scrambling
reshape folded into the final projection's lhsT layout). All matmul data
stays float32r (bf16 anywhere in the Q/K correlation path measurably breaks
the top-k selection: numpy study gives rel err 1.2e-2..3.4e-2 vs 5e-4).

Restructured for PE throughput vs the first working version (1.10 ms -> ~0.66 ms
neuron-profile device exec):
  - q/k loaded + transposed ONCE (the old version re-did both per channel
    group): the radix-2 time fold now happens on the RAW inputs (DVE adds,
    fold commutes with the projection), so folded spectra inputs come
    straight out of the projection and the PSUM fold drain disappears.
  - Q spectra land in SBUF; K spectra stay in PSUM and the spectral product
    overwrites the Q spectra in place (saves 32 PSUM->SBUF copies and all
    pool-lifetime conflicts nest LIFO as the tile framework requires).
  - The V path runs FIRST (it is independent): it warms the PE before the
    projections and parks projected V^T in DRAM, re-read at the roll phase,
    which frees 64KB/partition through the F/G/top-k phases.
  - PSUM drains alternate DVE/scalar; G-combine partner terms for odd t go
    to gpsimd; top-k(0) chunks are emitted between G(g1) quadrants so the
    DVE queue never blocks the G combines the PE waits on.
  - G accumulates the odd-k tiles first so their PSUM banks drain to SBUF
    while the even-k tiles accumulate, keeping all 8 banks cycling.

Sharding: data-parallel over batch B=8 -> one batch element per NeuronCore.
"""
import numpy as np

B, L, D, H, Dh = 8, 2048, 1024, 16, 64
KTOP = 7
NCORES = 8
P = 128

_prog_cache = {}


# ---------------------------------------------------------------- host helpers
def _round_fp32r(x):
    """Round fp32 to the ~13-bit-mantissa fp32r grid (matches measured HW)."""
    b = np.ascontiguousarray(x, dtype=np.float32).view(np.uint32)
    half = np.uint32(1 << 9)
    keep = np.uint32(0xFFFFFFFF) ^ np.uint32((1 << 10) - 1)
    out = ((b + half) & keep).view(np.float32)
    return np.where(np.isfinite(out), out, 0.0).astype(np.float32)


def _k_of(j, r):
    """k-index of packed row/col 128*j+r in the parity-permuted layout."""
    if j < 4:
        return 2 * (128 * j + r), "re"
    if j < 8:
        return 2 * (128 * (j - 4) + r) + 1, "re"
    if j < 12:
        return 2 * (128 * (j - 8) + r), "im"
    return 2 * (128 * (j - 12) + r) + 1, "im"


def _build_FG():
    """Radix-2-folded DFT matrices in the parity-permuted packed layout.

    Fh (1024, 2048): cols for even-k tiles multiply Qe = q[:1024]+q[1024:],
    odd-k tiles multiply Qo = q[:1024]-q[1024:]; sqrt(c_k/L) folded in.
    G (2048, 1024): rows give n in [0,1024); corr[n+1024] is recovered from
    the even-tile/odd-tile partial sums as A - B. Tile 8 row 0 carries the
    nyquist term (its natural occupant, im k=0, is identically zero)."""
    m = np.arange(1024)[:, None].astype(np.float64)
    n = np.arange(1024)[None, :].astype(np.float64)
    Fh = np.zeros((1024, 2048), dtype=np.float64)
    G = np.zeros((2048, 1024), dtype=np.float64)
    for j in range(16):
        for r in range(128):
            k, ri = _k_of(j, r)
            col = 128 * j + r
            if j == 8 and r == 0:  # nyquist slot
                Fh[:, col] = (np.cos(np.pi * m[:, 0]) * np.sqrt(1.0 / L))
                G[col, :] = np.cos(np.pi * n[0])
                continue
            ck = 1.0 if k == 0 else 2.0
            if ri == "re":
                Fh[:, col] = np.cos(2 * np.pi * m[:, 0] * k / L) * np.sqrt(ck / L)
                G[col, :] = np.cos(2 * np.pi * k * n[0] / L)
            else:
                Fh[:, col] = -np.sin(2 * np.pi * m[:, 0] * k / L) * np.sqrt(ck / L)
                G[col, :] = -np.sin(2 * np.pi * k * n[0] / L)
    return Fh.astype(np.float32), G.astype(np.float32)


def _tile_w(w):
    """(1024, 1024) -> (128, 8, 1024): [p, o, c] = w[o*128+p, c]."""
    return np.ascontiguousarray(
        w.reshape(8, P, D).transpose(1, 0, 2)).astype(np.float32)


def _host_constants():
    Fh, G = _build_FG()
    # fmat[t, p, mt] = [Fh[mt*P+p, t-block] | Fh[mt*P+p, (8+t)-block]]
    # (p-major so the per-t DMA into a [128, 8, 256] tile is contiguous)
    fmat = np.zeros((8, P, 8, 2 * P), dtype=np.float32)
    for t in range(8):
        for mt in range(8):
            fmat[t, :, mt, 0:P] = Fh[mt * P:(mt + 1) * P, t * P:(t + 1) * P]
            fmat[t, :, mt, P:2 * P] = Fh[mt * P:(mt + 1) * P,
                                         (8 + t) * P:(9 + t) * P]
    # gmat[kt, nn] = G[kt*128:(kt+1)*128, nn*512:(nn+1)*512], nn in 0..1
    gmat = np.zeros((16, 2, P, 512), dtype=np.float32)
    for kt in range(16):
        for nn in range(2):
            gmat[kt, nn] = G[kt * P:(kt + 1) * P, nn * 512:(nn + 1) * 512]
    return _round_fp32r(fmat), _round_fp32r(gmat)


# ---------------------------------------------------------------- wait splitting
def _split_excess_waits(nc, mybir):
    """walrus here allows max ONE sem wait per instruction; hoist extras onto
    injected NoOps on the same engine."""
    ctr = 0
    for f in nc.m.functions:
        for bblk in f.blocks:
            insts = bblk.instructions
            i = 0
            while i < len(insts):
                ins = insts[i]
                si = ins.sync_info
                if si is not None and len(si.on_wait) > 1:
                    waits = list(si.on_wait)
                    for w in waits[:-1]:
                        nop = mybir.InstNoOp(name=f"WSPL-{ctr}", ins=[], outs=[])
                        ctr += 1
                        nop.engine = ins.engine
                        nop.sync_info = mybir.SyncInfo(on_wait=[w], on_update=[])
                        insts.insert(i, nop)
                        i += 1
                    ins.sync_info = mybir.SyncInfo(
                        on_wait=[waits[-1]], on_update=list(si.on_update))
                i += 1


# ---------------------------------------------------------------- bass program
def _build_program():
    import concourse.bass as bass
    import concourse.mybir as mybir
    from concourse.tile import TileContext
    from concourse.masks import make_identity

    f32 = mybir.dt.float32
    f32r = mybir.dt.float32r
    i32 = mybir.dt.int32
    u32 = mybir.dt.uint32
    Alu = mybir.AluOpType

    nc = bass.Bass("TRN2", target_bir_lowering=False)

    qin = nc.dram_tensor("qin", (L, D), f32, kind="ExternalInput")
    kin = nc.dram_tensor("kin", (L, D), f32, kind="ExternalInput")
    vin = nc.dram_tensor("vin", (L, D), f32, kind="ExternalInput")
    wq_d = nc.dram_tensor("wq_t", (P, 8, D), f32r, kind="ExternalInput")
    wk_d = nc.dram_tensor("wk_t", (P, 8, D), f32r, kind="ExternalInput")
    wv_d = nc.dram_tensor("wv_t", (P, 8, D), f32r, kind="ExternalInput")
    wo_d = nc.dram_tensor("wo_t", (P, 8, D), f32r, kind="ExternalInput")
    f_d = nc.dram_tensor("fmat", (8, P, 8, 2 * P), f32r, kind="ExternalInput")
    g_d = nc.dram_tensor("gmat", (16, 2, P, 512), f32r, kind="ExternalInput")
    out_d = nc.dram_tensor("out", (L, D), f32, kind="ExternalOutput")
    vp_d = nc.dram_tensor("vproj_dram", (P, 8, L), f32r, kind="Internal")
    taps = {}
    if _prog_cache.get("debug_taps"):
        taps["feq"] = nc.dram_tensor("tap_feq", (2, P, 8, 512), f32r,
                                     kind="ExternalOutput")
        taps["foq"] = nc.dram_tensor("tap_foq", (2, P, 8, 512), f32r,
                                     kind="ExternalOutput")
        taps["sq"] = nc.dram_tensor("tap_sq", (2, P, 16, 512), f32r,
                                    kind="ExternalOutput")
        taps["sp"] = nc.dram_tensor("tap_sp", (2, P, 16, 512), f32r,
                                    kind="ExternalOutput")
        taps["corr"] = nc.dram_tensor("tap_corr", (2, P, 4, L), f32,
                                      kind="ExternalOutput")
        taps["m"] = nc.dram_tensor("tap_m", (P, 8, 64), f32,
                                   kind="ExternalOutput")
        taps["vp"] = nc.dram_tensor("tap_vp", (P, 8, L), f32,
                                    kind="ExternalOutput")
        taps["r2t"] = nc.dram_tensor("tap_r2t", (P, 8, 64, 32), f32r,
                                     kind="ExternalOutput")

    _rr = [0]

    def drain(dst, src_ap):
        # PSUM -> SBUF copy. f32r destinations must go through the DVE
        # (scalar/activation f32r output is not trustworthy); plain f32
        # copies alternate DVE/scalar to spread load.
        if _rr[0] % 2 == 0:
            nc.vector.tensor_copy(dst, src_ap)
        else:
            nc.scalar.copy(dst, src_ap)
        _rr[0] += 1

    with TileContext(nc) as tc:
        with tc.tile_pool(name="const", bufs=1) as cp:
            ident = cp.tile([P, P], f32)
            make_identity(nc, ident)
            # (128, 64) with 1 at (p, p % 64): a 64x64 identity at base 0 or 64
            ident2 = cp.tile([P, 64], f32)
            nc.gpsimd.memset(ident2, 0.0)
            nc.gpsimd.affine_select(
                out=ident2, in_=ident2, compare_op=Alu.not_equal, fill=1.0,
                base=0, channel_multiplier=1, pattern=[[-1, 64]])
            nc.gpsimd.affine_select(
                out=ident2, in_=ident2, compare_op=Alu.not_equal, fill=1.0,
                base=-64, channel_multiplier=1, pattern=[[-1, 64]])
            # T64f[p, s] = (p - s) & 63 as fp32
            t64i = cp.tile([P, 64], i32)
            nc.gpsimd.iota(t64i, pattern=[[-1, 64]], base=0, channel_multiplier=1)
            nc.vector.tensor_scalar(t64i, t64i, 63, None, op0=Alu.bitwise_and)
            t64f = cp.tile([P, 64], f32)
            nc.vector.tensor_copy(t64f, t64i)
            m_all = cp.tile([P, 8, 64], f32)    # roll matrices, (c, s)
            mp_all = cp.tile([P, 8, 64], f32r)  # transposed,  (s, t) per head
            smf = cp.tile([P, 8, 32], f32)      # per-cc top-k scratch
            smu = cp.tile([P, 8, 16], u32)
            tmp64a = cp.tile([P, 64], f32)
            tmp64b = cp.tile([P, 64], f32)

            # ------------- phase P/F: per tensor: fold -> transpose ->
            # project -> spectra.  Q spectra land in s_g; K spectra stay in
            # PSUM and the spectral product overwrites s_g in place.
            def proj_tensor(name, src_d, w_d):
                fep = tc.alloc_tile_pool(name=f"fe{name}", bufs=1)
                fe = [fep.tile([P, 8, 512], f32r, name=f"fe{name}{g}")
                      for g in range(2)]
                fo = [fep.tile([P, 8, 512], f32r, name=f"fo{name}{g}")
                      for g in range(2)]
                wpool = tc.alloc_tile_pool(name=f"w{name}", bufs=1)
                w_sb = wpool.tile([P, 8, D], f32r, name=f"w_{name}")
                for cc in range(8):
                    nc.sync.dma_start(w_sb[:, cc, :], w_d.ap()[:, cc, :])
                with tc.tile_pool(name=f"raw{name}", bufs=2) as rawp, \
                     tc.tile_pool(name=f"eo{name}", bufs=3) as eop, \
                     tc.tile_pool(name=f"xt{name}", bufs=3) as xtp, \
                     tc.tile_pool(name=f"trp{name}", bufs=2,
                                  space="PSUM") as trpp, \
                     tc.tile_pool(name=f"pp{name}", bufs=3,
                                  space="PSUM") as prjp:
                    for mc in range(8):
                        rlo = rawp.tile([P, D], f32, tag="rlo")
                        rhi = rawp.tile([P, D], f32, tag="rhi")
                        nc.sync.dma_start(rlo, src_d.ap()[mc * P:(mc + 1) * P, :])
                        nc.sync.dma_start(
                            rhi, src_d.ap()[(mc + 8) * P:(mc + 9) * P, :])
                        for br, op in ((0, Alu.add), (1, Alu.subtract)):
                            eo = eop.tile([P, D], f32, tag="eo",
                                          name=f"eo_{name}{mc}{br}")
                            nc.vector.tensor_tensor(eo, rlo, rhi, op)
                            # transpose eo -> xt (c-part, m) then project
                            xt = xtp.tile([P, 8, P], f32r, tag="xt",
                                          name=f"xt_{name}{mc}{br}")
                            for half in range(2):
                                trp = trpp.tile([P, 512], f32, tag="tr")
                                for j in range(4):
                                    ct = half * 4 + j
                                    nc.tensor.transpose(
                                        trp[:, j * P:(j + 1) * P],
                                        eo[:, ct * P:(ct + 1) * P], ident)
                                drain(xt[:, half * 4:(half + 1) * 4, :], trp)
                            dst = fe if br == 0 else fo
                            pr = prjp.tile([P, 2, 512], f32, tag="pr",
                                           name=f"pr_{name}{mc}{br}")
                            for g in range(2):
                                for cc in range(8):
                                    nc.tensor.matmul(
                                        pr[:, g, :], xt[:, cc, :],
                                        w_sb[:, cc, g * 512:(g + 1) * 512],
                                        start=(cc == 0), stop=(cc == 7))
                                drain(dst[g][:, mc, :], pr[:, g, :])
                wpool.release()
                return fep, fe, fo

            # ------------- phase V first (independent; warms the PE);
            # projected V^T goes to DRAM and is re-read at the roll phase
            wv_pool = tc.alloc_tile_pool(name="wv", bufs=1)
            wv_sb = wv_pool.tile([P, 8, D], f32r)
            with tc.tile_pool(name="vraw", bufs=3) as vrawp, \
                 tc.tile_pool(name="vxt", bufs=2) as vxtp, \
                 tc.tile_pool(name="vst", bufs=4) as vstp, \
                 tc.tile_pool(name="vtrp", bufs=2, space="PSUM") as vtrpp, \
                 tc.tile_pool(name="vpp", bufs=4, space="PSUM") as vpp:
                vxts = [None] * 4

                def v_transpose(mb):
                    vxt = vxtp.tile([P, 8, 512], f32r, tag="vxt",
                                    name=f"vxt_{mb}")
                    vxts[mb] = vxt
                    for mc in range(4):
                        raw = vrawp.tile([P, D], f32, tag="vraw")
                        nc.sync.dma_start(
                            raw, vin.ap()[(mb * 4 + mc) * P:
                                          (mb * 4 + mc + 1) * P, :])
                        for half in range(2):
                            trp = vtrpp.tile([P, 512], f32, tag="vtr")
                            for j in range(4):
                                ct = half * 4 + j
                                nc.tensor.transpose(
                                    trp[:, j * P:(j + 1) * P],
                                    raw[:, ct * P:(ct + 1) * P], ident)
                            drain(vxt[:, half * 4:(half + 1) * 4,
                                      mc * P:(mc + 1) * P], trp)

                def v_project(mb):
                    vxt = vxts[mb]
                    for cc8 in range(8):
                        pv = vpp.tile([P, 512], f32, tag="vpr")
                        for ct in range(8):
                            nc.tensor.matmul(
                                pv, wv_sb[:, ct, cc8 * P:(cc8 + 1) * P],
                                vxt[:, ct, :],
                                start=(ct == 0), stop=(ct == 7))
                        vst = vstp.tile([P, 512], f32r, tag="vst")
                        drain(vst, pv)
                        nc.sync.dma_start(
                            vp_d.ap()[:, cc8, mb * 512:(mb + 1) * 512], vst)

                v_transpose(0)
                for cc in range(8):
                    nc.sync.dma_start(wv_sb[:, cc, :], wv_d.ap()[:, cc, :])
                v_project(0)
                for mb in range(1, 4):
                    v_transpose(mb)
                    v_project(mb)
            wv_pool.release()

            s_pool = tc.alloc_tile_pool(name="sg", bufs=1)
            s_g = [s_pool.tile([P, 16, 512], f32r, name=f"s{g}")
                   for g in range(2)]

            # ---- Q: project + spectra into s_g
            fep_q, feq, foq = proj_tensor("q", qin, wq_d)
            with tc.tile_pool(name="ftiq", bufs=3) as ftp, \
                 tc.tile_pool(name="fpsq", bufs=3, space="PSUM") as fps:
                for g in range(2):
                    for t in range(8):
                        fti = ftp.tile([P, 8, 2 * P], f32r, tag="fti",
                                       name=f"ftiq_{g}_{t}")
                        nc.sync.dma_start(fti, f_d.ap()[t])
                        rhs_q = (feq if t < 4 else foq)[g]
                        pq = fps.tile([P, 2, 512], f32, tag="pq",
                                      name=f"pq_{g}_{t}")
                        for mt in range(8):
                            st, sp = (mt == 0), (mt == 7)
                            nc.tensor.matmul(pq[:, 0, :], fti[:, mt, 0:P],
                                             rhs_q[:, mt, :], start=st, stop=sp)
                            nc.tensor.matmul(pq[:, 1, :], fti[:, mt, P:2 * P],
                                             rhs_q[:, mt, :], start=st, stop=sp)
                        drain(s_g[g][:, t, :], pq[:, 0, :])
                        drain(s_g[g][:, 8 + t, :], pq[:, 1, :])
            if taps:
                for g in range(2):
                    nc.sync.dma_start(taps["feq"].ap()[g], feq[g])
                    nc.sync.dma_start(taps["foq"].ap()[g], foq[g])
                    nc.sync.dma_start(taps["sq"].ap()[g], s_g[g])
            fep_q.release()

            # ---- K: project + spectra; product overwrites s_g in place
            fep_k, fek, fok = proj_tensor("k", kin, wk_d)
            with tc.tile_pool(name="ftik", bufs=3) as ftp, \
                 tc.tile_pool(name="sppk", bufs=3) as spp, \
                 tc.tile_pool(name="fpsk", bufs=2, space="PSUM") as fps:
                for g in range(2):
                    for t in range(8):
                        fti = ftp.tile([P, 8, 2 * P], f32r, tag="fti",
                                       name=f"ftik_{g}_{t}")
                        nc.sync.dma_start(fti, f_d.ap()[t])
                        rhs_k = (fek if t < 4 else fok)[g]
                        pk = fps.tile([P, 2, 512], f32, tag="pk",
                                      name=f"pk_{g}_{t}")
                        for mt in range(8):
                            st, sp = (mt == 0), (mt == 7)
                            nc.tensor.matmul(pk[:, 0, :], fti[:, mt, 0:P],
                                             rhs_k[:, mt, :], start=st, stop=sp)
                            nc.tensor.matmul(pk[:, 1, :], fti[:, mt, P:2 * P],
                                             rhs_k[:, mt, :], start=st, stop=sp)
                        # spectral product: s_re = qre*kre + qim*kim,
                        # s_im = qim*kre - qre*kim (in-place over q spectra)
                        qre = s_g[g][:, t, :]
                        qim = s_g[g][:, 8 + t, :]
                        kre = pk[:, 0, :]
                        kim = pk[:, 1, :]
                        tm1 = spp.tile([P, 512], f32, tag="tm1")
                        tm2 = spp.tile([P, 512], f32, tag="tm2")
                        tm3 = spp.tile([P, 512], f32, tag="tm3")
                        tm4 = spp.tile([P, 512], f32, tag="tm4")
                        if t == 0:
                            # row 0 of tile 0 = DC (qre*kre); row 0 of tile 8
                            # = nyquist (qim*kim); compute before overwrite
                            dcny = spp.tile([1, 1024], f32, tag="dc",
                                            name=f"dcny_{g}")
                            nc.vector.tensor_tensor(
                                dcny[0:1, 0:512], qre[0:1, :], kre[0:1, :],
                                Alu.mult)
                            nc.vector.tensor_tensor(
                                dcny[0:1, 512:1024], qim[0:1, :], kim[0:1, :],
                                Alu.mult)
                        nc.vector.tensor_tensor(tm1, kre, qre, Alu.mult)
                        nc.vector.tensor_tensor(tm2, kim, qim, Alu.mult)
                        nc.vector.tensor_tensor(tm3, kre, qim, Alu.mult)
                        nc.vector.tensor_tensor(tm4, kim, qre, Alu.mult)
                        ceng = nc.vector if t % 2 == 0 else nc.gpsimd
                        ceng.tensor_tensor(qre, tm1, tm2, Alu.add)
                        ceng.tensor_tensor(qim, tm3, tm4, Alu.subtract)
                        if t == 0:
                            nc.vector.tensor_copy(s_g[g][0:1, 0, :],
                                                  dcny[0:1, 0:512])
                            nc.vector.tensor_copy(s_g[g][0:1, 8, :],
                                                  dcny[0:1, 512:1024])
            if taps:
                for g in range(2):
                    nc.sync.dma_start(taps["sp"].ap()[g], s_g[g])
            fep_k.release()

            def topk_chunk(g, c):
                if True:
                    gt_idx = g * 4 + c
                    topv = smf[:, gt_idx, 0:8]
                    expw = smf[:, gt_idx, 8:16]
                    shmf = smf[:, gt_idx, 16:24]
                    nv0 = smf[:, gt_idx, 24:25]
                    s7 = smf[:, gt_idx, 25:26]
                    r7 = smf[:, gt_idx, 26:27]
                    topi = smu[:, gt_idx, 0:8]
                    shmi = smu[:, gt_idx, 8:16]
                    nc.vector.max(out=topv, in_=corr[g][:, c, :])
                    nc.vector.max_index(out=topi, in_max=topv,
                                        in_values=corr[g][:, c, :])
                    nc.vector.tensor_scalar(nv0, topv[:, 0:1], -1.0, None,
                                            op0=Alu.mult)
                    nc.scalar.activation(
                        expw[:, 0:KTOP], topv[:, 0:KTOP],
                        mybir.ActivationFunctionType.Exp,
                        bias=nv0, scale=1.0)
                    nc.vector.reduce_sum(s7, expw[:, 0:KTOP],
                                         axis=mybir.AxisListType.X)
                    nc.vector.reciprocal(r7, s7)
                    nc.vector.tensor_scalar(expw[:, 0:KTOP], expw[:, 0:KTOP],
                                            r7, None, op0=Alu.mult)
                    nc.vector.tensor_scalar(shmi, topi, 63, None,
                                            op0=Alu.bitwise_and)
                    nc.vector.tensor_copy(shmf, shmi)
                    tmp64 = tmp64a if (c % 2 == 0) else tmp64b
                    for i in range(KTOP):
                        dst = m_all[:, gt_idx, :] if i == 0 else tmp64
                        nc.vector.tensor_scalar(
                            dst, t64f, shmf[:, i:i + 1], expw[:, i:i + 1],
                            op0=Alu.is_equal, op1=Alu.mult)
                        if i > 0:
                            nc.vector.tensor_tensor(
                                m_all[:, gt_idx, :], m_all[:, gt_idx, :],
                                tmp64, Alu.add)

            # ------------- phase G + top-k(0) interleaved
            corr_pool = tc.alloc_tile_pool(name="corr", bufs=1)
            corr = [corr_pool.tile([P, 4, L], f32, name=f"corr{g}")
                    for g in range(2)]

            ODD = (4, 5, 6, 7, 12, 13, 14, 15)
            EVEN = (0, 1, 2, 3, 8, 9, 10, 11)
            with tc.tile_pool(name="gt", bufs=6) as gtp, \
                 tc.tile_pool(name="pbs", bufs=1) as pbsp, \
                 tc.tile_pool(name="gps", bufs=1, space="PSUM") as gps:
                def g_quadrant(g, nn):
                    pB = [gps.tile([P, 512], f32, tag=f"B{c}",
                                   name=f"pB_{g}_{nn}_{c}")
                          for c in range(4)]
                    for kt in ODD:
                        gt = gtp.tile([P, 512], f32r, tag="gt",
                                      name=f"gt_{g}_{nn}_{kt}")
                        nc.sync.dma_start(gt, g_d.ap()[kt, nn])
                        for c in range(4):
                            nc.tensor.matmul(
                                pB[c], s_g[g][:, kt, c * P:(c + 1) * P],
                                gt, start=(kt == 4), stop=(kt == 15))
                    pBs = [pbsp.tile([P, 512], f32, tag=f"Bs{c}",
                                     name=f"pBs_{g}_{nn}_{c}")
                           for c in range(4)]
                    for c in range(4):
                        nc.scalar.copy(pBs[c], pB[c])
                    pA = [gps.tile([P, 512], f32, tag=f"A{c}",
                                   name=f"pA_{g}_{nn}_{c}")
                          for c in range(4)]
                    for kt in EVEN:
                        gt = gtp.tile([P, 512], f32r, tag="gt",
                                      name=f"gt_{g}_{nn}_{kt}")
                        nc.sync.dma_start(gt, g_d.ap()[kt, nn])
                        for c in range(4):
                            nc.tensor.matmul(
                                pA[c], s_g[g][:, kt, c * P:(c + 1) * P],
                                gt, start=(kt == 0), stop=(kt == 11))
                    for c in range(4):
                        nc.vector.tensor_tensor(
                            corr[g][:, c, nn * 512:(nn + 1) * 512],
                            pA[c], pBs[c], Alu.add)
                        nc.vector.tensor_tensor(
                            corr[g][:, c, 1024 + nn * 512:
                                    1024 + (nn + 1) * 512],
                            pA[c], pBs[c], Alu.subtract)

                g_quadrant(0, 0)
                g_quadrant(0, 1)
                g_quadrant(1, 0)
                topk_chunk(0, 0)
                topk_chunk(0, 1)
                g_quadrant(1, 1)
                topk_chunk(0, 2)
                topk_chunk(0, 3)

            if taps:
                for g in range(2):
                    nc.sync.dma_start(taps["corr"].ap()[g], corr[g])
            for c in range(4):
                topk_chunk(1, c)
            if taps:
                nc.sync.dma_start(taps["m"].ap(), m_all)
            corr_pool.release()
            s_pool.release()

            # ------------- roll + scramble + final projection
            wo_pool = tc.alloc_tile_pool(name="wo", bufs=1)
            wo_sb = wo_pool.tile([P, 8, D], f32r)
            for cc in range(8):
                nc.sync.dma_start(wo_sb[:, cc, :], wo_d.ap()[:, cc, :])
            r2t_pool = tc.alloc_tile_pool(name="r2t", bufs=1)
            r2t = r2t_pool.tile([P, 8, 64, 32], f32r)   # (p, kc, d, hp)

            with tc.tile_pool(name="vsin", bufs=2) as vsinp, \
                 tc.tile_pool(name="mtp", bufs=2, space="PSUM") as mtp, \
                 tc.tile_pool(name="rollp", bufs=6, space="PSUM") as rop:
                for h2 in range(8):
                    vsin = vsinp.tile([P, L], f32r, tag="vsin",
                                      name=f"vsin_{h2}")
                    nc.sync.dma_start(vsin, vp_d.ap()[:, h2, :])
                    for hh in range(2):
                        h = 2 * h2 + hh
                        pb = (h % 2) * 64
                        pm = mtp.tile([64, 64], f32, tag="mt")
                        nc.tensor.transpose(
                            pm, m_all[pb:pb + 64, h // 2, :],
                            ident2[pb:pb + 64, :])
                        nc.scalar.copy(mp_all[pb:pb + 64, h // 2, :], pm)
                        for kc in range(8):
                            pr = rop.tile([P, 2, 64], f32, tag="roll",
                                          name=f"pr_{h}_{kc}")
                            for pp_ in range(2):
                                lc = kc + 8 * pp_
                                nc.tensor.matmul(
                                    pr[:, pp_, :],
                                    vsin[pb:pb + 64, lc * P:(lc + 1) * P],
                                    mp_all[pb:pb + 64, h // 2, :],
                                    start=True, stop=True)
                            # heads 0-7: scalar only (DVE still on top-k);
                            # heads 8-15: alternate DVE/scalar
                            if h < 8 or (h * 8 + kc) % 2 == 1:
                                nc.scalar.copy(
                                    r2t[:, kc, :, h * 2:h * 2 + 2],
                                    pr.rearrange("p j d -> p d j"))
                            else:
                                nc.vector.tensor_copy(
                                    r2t[:, kc, :, h * 2:h * 2 + 2],
                                    pr.rearrange("p j d -> p d j"))
            if taps:
                nc.sync.dma_start(taps["r2t"].ap(), r2t)
            with tc.tile_pool(name="fpp", bufs=6, space="PSUM") as fpp, \
                 tc.tile_pool(name="osb", bufs=6) as osbp:
                for a in range(16):
                    for j2 in range(2):
                        pf = fpp.tile([P, 512], f32, tag="fin")
                        for kc in range(8):
                            nc.tensor.matmul(
                                pf, r2t[:, kc, 4 * a:4 * a + 4, :],
                                wo_sb[:, kc, j2 * 512:(j2 + 1) * 512],
                                start=(kc == 0), stop=(kc == 7))
                        osb = osbp.tile([P, 512], f32, tag="osb")
                        drain(osb, pf)
                        nc.sync.dma_start(
                            out_d.ap()[a * P:(a + 1) * P,
                                       j2 * 512:(j2 + 1) * 512], osb)
            r2t_pool.release()
            wo_pool.release()

    _split_excess_waits(nc, mybir)
    return nc


def _get_program():
    if "nc" not in _prog_cache:
        _prog_cache["nc"] = _build_program()
    return _prog_cache["nc"]


# ---------------------------------------------------------------- entry point
def _last_in_maps_get():
    return _prog_cache["last_in_maps"]


def kernel(queries, keys, values, wq, wk, wv, wo):
    from concourse.bass_utils import run_bass_kernel_spmd

    queries = np.ascontiguousarray(queries, np.float32)
    keys = np.ascontiguousarray(keys, np.float32)
    values = np.ascontiguousarray(values, np.float32)

    if "fg" not in _prog_cache:
        _prog_cache["fg"] = _host_constants()
    fmat, gmat = _prog_cache["fg"]
    consts = {
        "fmat": fmat, "gmat": gmat,
        "wq_t": _round_fp32r(_tile_w(np.asarray(wq, np.float32))),
        "wk_t": _round_fp32r(_tile_w(np.asarray(wk, np.float32))),
        "wv_t": _round_fp32r(_tile_w(np.asarray(wv, np.float32))),
        "wo_t": _round_fp32r(_tile_w(np.asarray(wo, np.float32))),
    }

    nc = _get_program()
    in_maps = []
    for b in range(NCORES):
        in_maps.append({
            "qin": np.ascontiguousarray(queries[b]),
            "kin": np.ascontiguousarray(keys[b]),
            "vin": np.ascontiguousarray(values[b]),
            **consts,
        })
    _prog_cache["last_in_maps"] = in_maps
    res = run_bass_kernel_spmd(nc, in_maps, core_ids=list(range(NCORES)),
                               trace=False)
    out = np.stack([res.results[b]["out"] for b in range(NCORES)], axis=0)
    return out.astype(np.float32)


# revision 41
# speedup vs baseline: 1.0819x; 1.0130x over previous
"""Trainium2 Bass kernel for nn_AutoCorrelation (Autoformer AutoCorrelation).

Math identical to the validated baseline (dense rfft-as-matmul with radix-2
time fold, DVE top-8, mod-64 roll as per-head 64x64 matmuls, scrambling
reshape folded into the final projection's lhsT layout). All matmul data
stays float32r (bf16 anywhere in the Q/K correlation path measurably breaks
the top-k selection: numpy study gives rel err 1.2e-2..3.4e-2 vs 5e-4).

Restructured for PE throughput vs the first working version (1.10 ms -> ~0.66 ms
neuron-profile device exec):
  - q/k loaded + transposed ONCE (the old version re-did both per channel
    group): the radix-2 time fold now happens on the RAW inputs (DVE adds,
    fold commutes with the projection), so folded spectra inputs come
    straight out of the projection and the PSUM fold drain disappears.
  - Q spectra land in SBUF; K spectra stay in PSUM and the spectral product
    overwrites the Q spectra in place (saves 32 PSUM->SBUF copies and all
    pool-lifetime conflicts nest LIFO as the tile framework requires).
  - The V path runs FIRST (it is independent): it warms the PE before the
    projections and parks projected V^T in DRAM, re-read at the roll phase,
    which frees 64KB/partition through the F/G/top-k phases.
  - PSUM drains alternate DVE/scalar; G-combine partner terms for odd t go
    to gpsimd; top-k(0) chunks are emitted between G(g1) quadrants so the
    DVE queue never blocks the G combines the PE waits on.
  - G accumulates the odd-k tiles first so their PSUM banks drain to SBUF
    while the even-k tiles accumulate, keeping all 8 banks cycling.

Sharding: data-parallel over batch B=8 -> one batch element per NeuronCore.
"""
import numpy as np

B, L, D, H, Dh = 8, 2048, 1024, 16, 64
KTOP = 7
NCORES = 8
P = 128

_prog_cache = {}


# ---------------------------------------------------------------- host helpers
def _round_fp32r(x):
    """Round fp32 to the ~13-bit-mantissa fp32r grid (matches measured HW)."""
    b = np.ascontiguousarray(x, dtype=np.float32).view(np.uint32)
    half = np.uint32(1 << 9)
    keep = np.uint32(0xFFFFFFFF) ^ np.uint32((1 << 10) - 1)
    out = ((b + half) & keep).view(np.float32)
    return np.where(np.isfinite(out), out, 0.0).astype(np.float32)


def _k_of(j, r):
    """k-index of packed row/col 128*j+r in the parity-permuted layout."""
    if j < 4:
        return 2 * (128 * j + r), "re"
    if j < 8:
        return 2 * (128 * (j - 4) + r) + 1, "re"
    if j < 12:
        return 2 * (128 * (j - 8) + r), "im"
    return 2 * (128 * (j - 12) + r) + 1, "im"


def _build_FG():
    """Radix-2-folded DFT matrices in the parity-permuted packed layout.

    Fh (1024, 2048): cols for even-k tiles multiply Qe = q[:1024]+q[1024:],
    odd-k tiles multiply Qo = q[:1024]-q[1024:]; sqrt(c_k/L) folded in.
    G (2048, 1024): rows give n in [0,1024); corr[n+1024] is recovered from
    the even-tile/odd-tile partial sums as A - B. Tile 8 row 0 carries the
    nyquist term (its natural occupant, im k=0, is identically zero)."""
    m = np.arange(1024)[:, None].astype(np.float64)
    n = np.arange(1024)[None, :].astype(np.float64)
    Fh = np.zeros((1024, 2048), dtype=np.float64)
    G = np.zeros((2048, 1024), dtype=np.float64)
    for j in range(16):
        for r in range(128):
            k, ri = _k_of(j, r)
            col = 128 * j + r
            if j == 8 and r == 0:  # nyquist slot
                Fh[:, col] = (np.cos(np.pi * m[:, 0]) * np.sqrt(1.0 / L))
                G[col, :] = np.cos(np.pi * n[0])
                continue
            ck = 1.0 if k == 0 else 2.0
            if ri == "re":
                Fh[:, col] = np.cos(2 * np.pi * m[:, 0] * k / L) * np.sqrt(ck / L)
                G[col, :] = np.cos(2 * np.pi * k * n[0] / L)
            else:
                Fh[:, col] = -np.sin(2 * np.pi * m[:, 0] * k / L) * np.sqrt(ck / L)
                G[col, :] = -np.sin(2 * np.pi * k * n[0] / L)
    return Fh.astype(np.float32), G.astype(np.float32)


def _tile_w(w):
    """(1024, 1024) -> (128, 8, 1024): [p, o, c] = w[o*128+p, c]."""
    return np.ascontiguousarray(
        w.reshape(8, P, D).transpose(1, 0, 2)).astype(np.float32)


def _host_constants():
    Fh, G = _build_FG()
    # fmat[t, p, mt] = [Fh[mt*P+p, t-block] | Fh[mt*P+p, (8+t)-block]]
    # (p-major so the per-t DMA into a [128, 8, 256] tile is contiguous)
    fmat = np.zeros((8, P, 8, 2 * P), dtype=np.float32)
    for t in range(8):
        for mt in range(8):
            fmat[t, :, mt, 0:P] = Fh[mt * P:(mt + 1) * P, t * P:(t + 1) * P]
            fmat[t, :, mt, P:2 * P] = Fh[mt * P:(mt + 1) * P,
                                         (8 + t) * P:(9 + t) * P]
    # gmat[kt, nn] = G[kt*128:(kt+1)*128, nn*512:(nn+1)*512], nn in 0..1
    gmat = np.zeros((16, 2, P, 512), dtype=np.float32)
    for kt in range(16):
        for nn in range(2):
            gmat[kt, nn] = G[kt * P:(kt + 1) * P, nn * 512:(nn + 1) * 512]
    return _round_fp32r(fmat), _round_fp32r(gmat)


# ---------------------------------------------------------------- wait splitting
def _split_excess_waits(nc, mybir):
    """walrus here allows max ONE sem wait per instruction; hoist extras onto
    injected NoOps on the same engine."""
    ctr = 0
    for f in nc.m.functions:
        for bblk in f.blocks:
            insts = bblk.instructions
            i = 0
            while i < len(insts):
                ins = insts[i]
                si = ins.sync_info
                if si is not None and len(si.on_wait) > 1:
                    waits = list(si.on_wait)
                    for w in waits[:-1]:
                        nop = mybir.InstNoOp(name=f"WSPL-{ctr}", ins=[], outs=[])
                        ctr += 1
                        nop.engine = ins.engine
                        nop.sync_info = mybir.SyncInfo(on_wait=[w], on_update=[])
                        insts.insert(i, nop)
                        i += 1
                    ins.sync_info = mybir.SyncInfo(
                        on_wait=[waits[-1]], on_update=list(si.on_update))
                i += 1


# ---------------------------------------------------------------- bass program
def _build_program():
    import concourse.bass as bass
    import concourse.mybir as mybir
    from concourse.tile import TileContext
    from concourse.masks import make_identity

    f32 = mybir.dt.float32
    f32r = mybir.dt.float32r
    i32 = mybir.dt.int32
    u32 = mybir.dt.uint32
    Alu = mybir.AluOpType

    nc = bass.Bass("TRN2", target_bir_lowering=False)

    qin = nc.dram_tensor("qin", (L, D), f32, kind="ExternalInput")
    kin = nc.dram_tensor("kin", (L, D), f32, kind="ExternalInput")
    vin = nc.dram_tensor("vin", (L, D), f32, kind="ExternalInput")
    wq_d = nc.dram_tensor("wq_t", (P, 8, D), f32r, kind="ExternalInput")
    wk_d = nc.dram_tensor("wk_t", (P, 8, D), f32r, kind="ExternalInput")
    wv_d = nc.dram_tensor("wv_t", (P, 8, D), f32r, kind="ExternalInput")
    wo_d = nc.dram_tensor("wo_t", (P, 8, D), f32r, kind="ExternalInput")
    f_d = nc.dram_tensor("fmat", (8, P, 8, 2 * P), f32r, kind="ExternalInput")
    g_d = nc.dram_tensor("gmat", (16, 2, P, 512), f32r, kind="ExternalInput")
    out_d = nc.dram_tensor("out", (L, D), f32, kind="ExternalOutput")
    vp_d = nc.dram_tensor("vproj_dram", (P, 8, L), f32r, kind="Internal")
    taps = {}
    if _prog_cache.get("debug_taps"):
        taps["feq"] = nc.dram_tensor("tap_feq", (2, P, 8, 512), f32r,
                                     kind="ExternalOutput")
        taps["foq"] = nc.dram_tensor("tap_foq", (2, P, 8, 512), f32r,
                                     kind="ExternalOutput")
        taps["sq"] = nc.dram_tensor("tap_sq", (2, P, 16, 512), f32r,
                                    kind="ExternalOutput")
        taps["sp"] = nc.dram_tensor("tap_sp", (2, P, 16, 512), f32r,
                                    kind="ExternalOutput")
        taps["corr"] = nc.dram_tensor("tap_corr", (2, P, 4, L), f32,
                                      kind="ExternalOutput")
        taps["m"] = nc.dram_tensor("tap_m", (P, 8, 64), f32,
                                   kind="ExternalOutput")
        taps["vp"] = nc.dram_tensor("tap_vp", (P, 8, L), f32,
                                    kind="ExternalOutput")
        taps["r2t"] = nc.dram_tensor("tap_r2t", (P, 8, 64, 32), f32r,
                                     kind="ExternalOutput")

    _rr = [0]

    def drain(dst, src_ap):
        # PSUM -> SBUF copy. f32r destinations must go through the DVE
        # (scalar/activation f32r output is not trustworthy); plain f32
        # copies alternate DVE/scalar to spread load.
        if _rr[0] % 2 == 0:
            nc.vector.tensor_copy(dst, src_ap)
        else:
            nc.scalar.copy(dst, src_ap)
        _rr[0] += 1

    with TileContext(nc) as tc:
        with tc.tile_pool(name="const", bufs=1) as cp:
            ident = cp.tile([P, P], f32)
            make_identity(nc, ident)
            # (128, 64) with 1 at (p, p % 64): a 64x64 identity at base 0 or 64
            ident2 = cp.tile([P, 64], f32)
            nc.gpsimd.memset(ident2, 0.0)
            nc.gpsimd.affine_select(
                out=ident2, in_=ident2, compare_op=Alu.not_equal, fill=1.0,
                base=0, channel_multiplier=1, pattern=[[-1, 64]])
            nc.gpsimd.affine_select(
                out=ident2, in_=ident2, compare_op=Alu.not_equal, fill=1.0,
                base=-64, channel_multiplier=1, pattern=[[-1, 64]])
            # T64f[p, s] = (p - s) & 63 as fp32
            t64i = cp.tile([P, 64], i32)
            nc.gpsimd.iota(t64i, pattern=[[-1, 64]], base=0, channel_multiplier=1)
            nc.vector.tensor_scalar(t64i, t64i, 63, None, op0=Alu.bitwise_and)
            t64f = cp.tile([P, 64], f32)
            nc.vector.tensor_copy(t64f, t64i)
            m_all = cp.tile([P, 8, 64], f32)    # roll matrices, (c, s)
            mp_all = cp.tile([P, 8, 64], f32r)  # transposed,  (s, t) per head
            smf = cp.tile([P, 8, 32], f32)      # per-cc top-k scratch
            smu = cp.tile([P, 8, 16], u32)
            tmp64a = cp.tile([P, 64], f32)
            tmp64b = cp.tile([P, 64], f32)

            # ------------- phase P/F: per tensor: fold -> transpose ->
            # project -> spectra.  Q spectra land in s_g; K spectra stay in
            # PSUM and the spectral product overwrites s_g in place.
            def proj_tensor(name, src_d, w_d):
                fep = tc.alloc_tile_pool(name=f"fe{name}", bufs=1)
                fe = [fep.tile([P, 8, 512], f32r, name=f"fe{name}{g}")
                      for g in range(2)]
                fo = [fep.tile([P, 8, 512], f32r, name=f"fo{name}{g}")
                      for g in range(2)]
                wpool = tc.alloc_tile_pool(name=f"w{name}", bufs=1)
                w_sb = wpool.tile([P, 8, D], f32r, name=f"w_{name}")
                for cc in range(8):
                    nc.sync.dma_start(w_sb[:, cc, :], w_d.ap()[:, cc, :])
                with tc.tile_pool(name=f"raw{name}", bufs=2) as rawp, \
                     tc.tile_pool(name=f"eo{name}", bufs=3) as eop, \
                     tc.tile_pool(name=f"xt{name}", bufs=3) as xtp, \
                     tc.tile_pool(name=f"trp{name}", bufs=2,
                                  space="PSUM") as trpp, \
                     tc.tile_pool(name=f"pp{name}", bufs=3,
                                  space="PSUM") as prjp:
                    for mc in range(8):
                        rlo = rawp.tile([P, D], f32, tag="rlo")
                        rhi = rawp.tile([P, D], f32, tag="rhi")
                        nc.sync.dma_start(rlo, src_d.ap()[mc * P:(mc + 1) * P, :])
                        nc.sync.dma_start(
                            rhi, src_d.ap()[(mc + 8) * P:(mc + 9) * P, :])
                        for br, op in ((0, Alu.add), (1, Alu.subtract)):
                            eo = eop.tile([P, D], f32, tag="eo",
                                          name=f"eo_{name}{mc}{br}")
                            nc.vector.tensor_tensor(eo, rlo, rhi, op)
                            # transpose eo -> xt (c-part, m) then project
                            xt = xtp.tile([P, 8, P], f32r, tag="xt",
                                          name=f"xt_{name}{mc}{br}")
                            for half in range(2):
                                trp = trpp.tile([P, 512], f32, tag="tr")
                                for j in range(4):
                                    ct = half * 4 + j
                                    nc.tensor.transpose(
                                        trp[:, j * P:(j + 1) * P],
                                        eo[:, ct * P:(ct + 1) * P], ident)
                                drain(xt[:, half * 4:(half + 1) * 4, :], trp)
                            dst = fe if br == 0 else fo
                            pr = prjp.tile([P, 2, 512], f32, tag="pr",
                                           name=f"pr_{name}{mc}{br}")
                            for g in range(2):
                                for cc in range(8):
                                    nc.tensor.matmul(
                                        pr[:, g, :], xt[:, cc, :],
                                        w_sb[:, cc, g * 512:(g + 1) * 512],
                                        start=(cc == 0), stop=(cc == 7))
                                drain(dst[g][:, mc, :], pr[:, g, :])
                wpool.release()
                return fep, fe, fo

            # ------------- phase V first (independent; warms the PE);
            # projected V^T goes to DRAM and is re-read at the roll phase
            wv_pool = tc.alloc_tile_pool(name="wv", bufs=1)
            wv_sb = wv_pool.tile([P, 8, D], f32r)
            with tc.tile_pool(name="vraw", bufs=3) as vrawp, \
                 tc.tile_pool(name="vxt", bufs=2) as vxtp, \
                 tc.tile_pool(name="vst", bufs=6) as vstp, \
                 tc.tile_pool(name="vtrp", bufs=2, space="PSUM") as vtrpp, \
                 tc.tile_pool(name="vpp", bufs=6, space="PSUM") as vpp:
                vxts = [None] * 4

                def v_transpose(mb):
                    vxt = vxtp.tile([P, 8, 512], f32r, tag="vxt",
                                    name=f"vxt_{mb}")
                    vxts[mb] = vxt
                    for mc in range(4):
                        raw = vrawp.tile([P, D], f32, tag="vraw")
                        nc.sync.dma_start(
                            raw, vin.ap()[(mb * 4 + mc) * P:
                                          (mb * 4 + mc + 1) * P, :])
                        for half in range(2):
                            trp = vtrpp.tile([P, 512], f32, tag="vtr")
                            for j in range(4):
                                ct = half * 4 + j
                                nc.tensor.transpose(
                                    trp[:, j * P:(j + 1) * P],
                                    raw[:, ct * P:(ct + 1) * P], ident)
                            drain(vxt[:, half * 4:(half + 1) * 4,
                                      mc * P:(mc + 1) * P], trp)

                def v_project(mb):
                    vxt = vxts[mb]
                    for cc8 in range(8):
                        pv = vpp.tile([P, 512], f32, tag="vpr")
                        for ct in range(8):
                            nc.tensor.matmul(
                                pv, wv_sb[:, ct, cc8 * P:(cc8 + 1) * P],
                                vxt[:, ct, :],
                                start=(ct == 0), stop=(ct == 7))
                        vst = vstp.tile([P, 512], f32r, tag="vst")
                        drain(vst, pv)
                        nc.sync.dma_start(
                            vp_d.ap()[:, cc8, mb * 512:(mb + 1) * 512], vst)

                v_transpose(0)
                for cc in range(8):
                    nc.sync.dma_start(wv_sb[:, cc, :], wv_d.ap()[:, cc, :])
                v_project(0)
                for mb in range(1, 4):
                    v_transpose(mb)
                    v_project(mb)
            wv_pool.release()

            s_pool = tc.alloc_tile_pool(name="sg", bufs=1)
            s_g = [s_pool.tile([P, 16, 512], f32r, name=f"s{g}")
                   for g in range(2)]

            # ---- Q: project + spectra into s_g
            fep_q, feq, foq = proj_tensor("q", qin, wq_d)
            with tc.tile_pool(name="ftiq", bufs=3) as ftp, \
                 tc.tile_pool(name="fpsq", bufs=4, space="PSUM") as fps:
                for g in range(2):
                    for t in range(8):
                        fti = ftp.tile([P, 8, 2 * P], f32r, tag="fti",
                                       name=f"ftiq_{g}_{t}")
                        nc.sync.dma_start(fti, f_d.ap()[t])
                        rhs_q = (feq if t < 4 else foq)[g]
                        pq = fps.tile([P, 2, 512], f32, tag="pq",
                                      name=f"pq_{g}_{t}")
                        for mt in range(8):
                            st, sp = (mt == 0), (mt == 7)
                            nc.tensor.matmul(pq[:, 0, :], fti[:, mt, 0:P],
                                             rhs_q[:, mt, :], start=st, stop=sp)
                            nc.tensor.matmul(pq[:, 1, :], fti[:, mt, P:2 * P],
                                             rhs_q[:, mt, :], start=st, stop=sp)
                        drain(s_g[g][:, t, :], pq[:, 0, :])
                        drain(s_g[g][:, 8 + t, :], pq[:, 1, :])
            if taps:
                for g in range(2):
                    nc.sync.dma_start(taps["feq"].ap()[g], feq[g])
                    nc.sync.dma_start(taps["foq"].ap()[g], foq[g])
                    nc.sync.dma_start(taps["sq"].ap()[g], s_g[g])
            fep_q.release()

            # ---- K: project + spectra; product overwrites s_g in place
            fep_k, fek, fok = proj_tensor("k", kin, wk_d)
            with tc.tile_pool(name="ftik", bufs=3) as ftp, \
                 tc.tile_pool(name="sppk", bufs=3) as spp, \
                 tc.tile_pool(name="fpsk", bufs=2, space="PSUM") as fps:
                for g in range(2):
                    for t in range(8):
                        fti = ftp.tile([P, 8, 2 * P], f32r, tag="fti",
                                       name=f"ftik_{g}_{t}")
                        nc.sync.dma_start(fti, f_d.ap()[t])
                        rhs_k = (fek if t < 4 else fok)[g]
                        pk = fps.tile([P, 2, 512], f32, tag="pk",
                                      name=f"pk_{g}_{t}")
                        for mt in range(8):
                            st, sp = (mt == 0), (mt == 7)
                            nc.tensor.matmul(pk[:, 0, :], fti[:, mt, 0:P],
                                             rhs_k[:, mt, :], start=st, stop=sp)
                            nc.tensor.matmul(pk[:, 1, :], fti[:, mt, P:2 * P],
                                             rhs_k[:, mt, :], start=st, stop=sp)
                        # spectral product: s_re = qre*kre + qim*kim,
                        # s_im = qim*kre - qre*kim (in-place over q spectra)
                        qre = s_g[g][:, t, :]
                        qim = s_g[g][:, 8 + t, :]
                        kre = pk[:, 0, :]
                        kim = pk[:, 1, :]
                        tm1 = spp.tile([P, 512], f32, tag="tm1")
                        tm2 = spp.tile([P, 512], f32, tag="tm2")
                        tm3 = spp.tile([P, 512], f32, tag="tm3")
                        tm4 = spp.tile([P, 512], f32, tag="tm4")
                        if t == 0:
                            # row 0 of tile 0 = DC (qre*kre); row 0 of tile 8
                            # = nyquist (qim*kim); compute before overwrite
                            dcny = spp.tile([1, 1024], f32, tag="dc",
                                            name=f"dcny_{g}")
                            nc.vector.tensor_tensor(
                                dcny[0:1, 0:512], qre[0:1, :], kre[0:1, :],
                                Alu.mult)
                            nc.vector.tensor_tensor(
                                dcny[0:1, 512:1024], qim[0:1, :], kim[0:1, :],
                                Alu.mult)
                        nc.vector.tensor_tensor(tm1, kre, qre, Alu.mult)
                        nc.vector.tensor_tensor(tm2, kim, qim, Alu.mult)
                        nc.vector.tensor_tensor(tm3, kre, qim, Alu.mult)
                        nc.vector.tensor_tensor(tm4, kim, qre, Alu.mult)
                        ceng = nc.vector if t % 2 == 0 else nc.gpsimd
                        ceng.tensor_tensor(qre, tm1, tm2, Alu.add)
                        ceng.tensor_tensor(qim, tm3, tm4, Alu.subtract)
                        if t == 0:
                            nc.vector.tensor_copy(s_g[g][0:1, 0, :],
                                                  dcny[0:1, 0:512])
                            nc.vector.tensor_copy(s_g[g][0:1, 8, :],
                                                  dcny[0:1, 512:1024])
            if taps:
                for g in range(2):
                    nc.sync.dma_start(taps["sp"].ap()[g], s_g[g])
            fep_k.release()

            def topk_chunk(g, c):
                if True:
                    gt_idx = g * 4 + c
                    topv = smf[:, gt_idx, 0:8]
                    expw = smf[:, gt_idx, 8:16]
                    shmf = smf[:, gt_idx, 16:24]
                    nv0 = smf[:, gt_idx, 24:25]
                    s7 = smf[:, gt_idx, 25:26]
                    r7 = smf[:, gt_idx, 26:27]
                    topi = smu[:, gt_idx, 0:8]
                    shmi = smu[:, gt_idx, 8:16]
                    nc.vector.max(out=topv, in_=corr[g][:, c, :])
                    nc.vector.max_index(out=topi, in_max=topv,
                                        in_values=corr[g][:, c, :])
                    nc.vector.tensor_scalar(nv0, topv[:, 0:1], -1.0, None,
                                            op0=Alu.mult)
                    nc.scalar.activation(
                        expw[:, 0:KTOP], topv[:, 0:KTOP],
                        mybir.ActivationFunctionType.Exp,
                        bias=nv0, scale=1.0)
                    nc.vector.reduce_sum(s7, expw[:, 0:KTOP],
                                         axis=mybir.AxisListType.X)
                    nc.vector.reciprocal(r7, s7)
                    nc.vector.tensor_scalar(expw[:, 0:KTOP], expw[:, 0:KTOP],
                                            r7, None, op0=Alu.mult)
                    nc.vector.tensor_scalar(shmi, topi, 63, None,
                                            op0=Alu.bitwise_and)
                    nc.vector.tensor_copy(shmf, shmi)
                    tmp64 = tmp64a if (c % 2 == 0) else tmp64b
                    for i in range(KTOP):
                        dst = m_all[:, gt_idx, :] if i == 0 else tmp64
                        nc.vector.tensor_scalar(
                            dst, t64f, shmf[:, i:i + 1], expw[:, i:i + 1],
                            op0=Alu.is_equal, op1=Alu.mult)
                        if i > 0:
                            nc.vector.tensor_tensor(
                                m_all[:, gt_idx, :], m_all[:, gt_idx, :],
                                tmp64, Alu.add)

            # ------------- phase G + top-k(0) interleaved
            corr_pool = tc.alloc_tile_pool(name="corr", bufs=1)
            corr = [corr_pool.tile([P, 4, L], f32, name=f"corr{g}")
                    for g in range(2)]

            ODD = (4, 5, 6, 7, 12, 13, 14, 15)
            EVEN = (0, 1, 2, 3, 8, 9, 10, 11)
            with tc.tile_pool(name="gt", bufs=6) as gtp, \
                 tc.tile_pool(name="pbs", bufs=1) as pbsp, \
                 tc.tile_pool(name="gps", bufs=1, space="PSUM") as gps:
                def g_quadrant(g, nn):
                    pB = [gps.tile([P, 512], f32, tag=f"B{c}",
                                   name=f"pB_{g}_{nn}_{c}")
                          for c in range(4)]
                    for kt in ODD:
                        gt = gtp.tile([P, 512], f32r, tag="gt",
                                      name=f"gt_{g}_{nn}_{kt}")
                        nc.sync.dma_start(gt, g_d.ap()[kt, nn])
                        for c in range(4):
                            nc.tensor.matmul(
                                pB[c], s_g[g][:, kt, c * P:(c + 1) * P],
                                gt, start=(kt == 4), stop=(kt == 15))
                    pBs = [pbsp.tile([P, 512], f32, tag=f"Bs{c}",
                                     name=f"pBs_{g}_{nn}_{c}")
                           for c in range(4)]
                    for c in range(4):
                        nc.scalar.copy(pBs[c], pB[c])
                    pA = [gps.tile([P, 512], f32, tag=f"A{c}",
                                   name=f"pA_{g}_{nn}_{c}")
                          for c in range(4)]
                    for kt in EVEN:
                        gt = gtp.tile([P, 512], f32r, tag="gt",
                                      name=f"gt_{g}_{nn}_{kt}")
                        nc.sync.dma_start(gt, g_d.ap()[kt, nn])
                        for c in range(4):
                            nc.tensor.matmul(
                                pA[c], s_g[g][:, kt, c * P:(c + 1) * P],
                                gt, start=(kt == 0), stop=(kt == 11))
                    for c in range(4):
                        nc.vector.tensor_tensor(
                            corr[g][:, c, nn * 512:(nn + 1) * 512],
                            pA[c], pBs[c], Alu.add)
                        nc.vector.tensor_tensor(
                            corr[g][:, c, 1024 + nn * 512:
                                    1024 + (nn + 1) * 512],
                            pA[c], pBs[c], Alu.subtract)

                g_quadrant(0, 0)
                g_quadrant(0, 1)
                g_quadrant(1, 0)
                topk_chunk(0, 0)
                topk_chunk(0, 1)
                g_quadrant(1, 1)
                topk_chunk(0, 2)
                topk_chunk(0, 3)

            if taps:
                for g in range(2):
                    nc.sync.dma_start(taps["corr"].ap()[g], corr[g])
            for c in range(4):
                topk_chunk(1, c)
            if taps:
                nc.sync.dma_start(taps["m"].ap(), m_all)
            corr_pool.release()
            s_pool.release()

            # ------------- roll + scramble + final projection
            wo_pool = tc.alloc_tile_pool(name="wo", bufs=1)
            wo_sb = wo_pool.tile([P, 8, D], f32r)
            for cc in range(8):
                nc.sync.dma_start(wo_sb[:, cc, :], wo_d.ap()[:, cc, :])
            r2t_pool = tc.alloc_tile_pool(name="r2t", bufs=1)
            r2t = r2t_pool.tile([P, 8, 64, 32], f32r)   # (p, kc, d, hp)

            with tc.tile_pool(name="vsin", bufs=3) as vsinp, \
                 tc.tile_pool(name="mtp", bufs=2, space="PSUM") as mtp, \
                 tc.tile_pool(name="rollp", bufs=6, space="PSUM") as rop:
                for h2 in range(8):
                    vsin = vsinp.tile([P, L], f32r, tag="vsin",
                                      name=f"vsin_{h2}")
                    nc.sync.dma_start(vsin, vp_d.ap()[:, h2, :])
                    for hh in range(2):
                        h = 2 * h2 + hh
                        pb = (h % 2) * 64
                        pm = mtp.tile([64, 64], f32, tag="mt")
                        nc.tensor.transpose(
                            pm, m_all[pb:pb + 64, h // 2, :],
                            ident2[pb:pb + 64, :])
                        nc.scalar.copy(mp_all[pb:pb + 64, h // 2, :], pm)
                        for kc in range(8):
                            pr = rop.tile([P, 2, 64], f32, tag="roll",
                                          name=f"pr_{h}_{kc}")
                            for pp_ in range(2):
                                lc = kc + 8 * pp_
                                nc.tensor.matmul(
                                    pr[:, pp_, :],
                                    vsin[pb:pb + 64, lc * P:(lc + 1) * P],
                                    mp_all[pb:pb + 64, h // 2, :],
                                    start=True, stop=True)
                            # heads 0-7: scalar only (DVE still on top-k);
                            # heads 8-15: alternate DVE/scalar
                            if h < 8 or (h * 8 + kc) % 2 == 1:
                                nc.scalar.copy(
                                    r2t[:, kc, :, h * 2:h * 2 + 2],
                                    pr.rearrange("p j d -> p d j"))
                            else:
                                nc.vector.tensor_copy(
                                    r2t[:, kc, :, h * 2:h * 2 + 2],
                                    pr.rearrange("p j d -> p d j"))
            if taps:
                nc.sync.dma_start(taps["r2t"].ap(), r2t)
            with tc.tile_pool(name="fpp", bufs=6, space="PSUM") as fpp, \
                 tc.tile_pool(name="osb", bufs=6) as osbp:
                for a in range(16):
                    for j2 in range(2):
                        pf = fpp.tile([P, 512], f32, tag="fin")
                        for kc in range(8):
                            nc.tensor.matmul(
                                pf, r2t[:, kc, 4 * a:4 * a + 4, :],
                                wo_sb[:, kc, j2 * 512:(j2 + 1) * 512],
                                start=(kc == 0), stop=(kc == 7))
                        osb = osbp.tile([P, 512], f32, tag="osb")
                        drain(osb, pf)
                        nc.sync.dma_start(
                            out_d.ap()[a * P:(a + 1) * P,
                                       j2 * 512:(j2 + 1) * 512], osb)
            r2t_pool.release()
            wo_pool.release()

    _split_excess_waits(nc, mybir)
    return nc


def _get_program():
    if "nc" not in _prog_cache:
        _prog_cache["nc"] = _build_program()
    return _prog_cache["nc"]


# ---------------------------------------------------------------- entry point
def _last_in_maps_get():
    return _prog_cache["last_in_maps"]


def kernel(queries, keys, values, wq, wk, wv, wo):
    from concourse.bass_utils import run_bass_kernel_spmd

    queries = np.ascontiguousarray(queries, np.float32)
    keys = np.ascontiguousarray(keys, np.float32)
    values = np.ascontiguousarray(values, np.float32)

    if "fg" not in _prog_cache:
        _prog_cache["fg"] = _host_constants()
    fmat, gmat = _prog_cache["fg"]
    consts = {
        "fmat": fmat, "gmat": gmat,
        "wq_t": _round_fp32r(_tile_w(np.asarray(wq, np.float32))),
        "wk_t": _round_fp32r(_tile_w(np.asarray(wk, np.float32))),
        "wv_t": _round_fp32r(_tile_w(np.asarray(wv, np.float32))),
        "wo_t": _round_fp32r(_tile_w(np.asarray(wo, np.float32))),
    }

    nc = _get_program()
    in_maps = []
    for b in range(NCORES):
        in_maps.append({
            "qin": np.ascontiguousarray(queries[b]),
            "kin": np.ascontiguousarray(keys[b]),
            "vin": np.ascontiguousarray(values[b]),
            **consts,
        })
    _prog_cache["last_in_maps"] = in_maps
    res = run_bass_kernel_spmd(nc, in_maps, core_ids=list(range(NCORES)),
                               trace=False)
    out = np.stack([res.results[b]["out"] for b in range(NCORES)], axis=0)
    return out.astype(np.float32)


# revision 42
# speedup vs baseline: 1.0823x; 1.0004x over previous
"""Trainium2 Bass kernel for nn_AutoCorrelation (Autoformer AutoCorrelation).

Math identical to the validated baseline (dense rfft-as-matmul with radix-2
time fold, DVE top-8, mod-64 roll as per-head 64x64 matmuls, scrambling
reshape folded into the final projection's lhsT layout). All matmul data
stays float32r (bf16 anywhere in the Q/K correlation path measurably breaks
the top-k selection: numpy study gives rel err 1.2e-2..3.4e-2 vs 5e-4).

Restructured for PE throughput vs the first working version (1.10 ms -> ~0.66 ms
neuron-profile device exec):
  - q/k loaded + transposed ONCE (the old version re-did both per channel
    group): the radix-2 time fold now happens on the RAW inputs (DVE adds,
    fold commutes with the projection), so folded spectra inputs come
    straight out of the projection and the PSUM fold drain disappears.
  - Q spectra land in SBUF; K spectra stay in PSUM and the spectral product
    overwrites the Q spectra in place (saves 32 PSUM->SBUF copies and all
    pool-lifetime conflicts nest LIFO as the tile framework requires).
  - The V path runs FIRST (it is independent): it warms the PE before the
    projections and parks projected V^T in DRAM, re-read at the roll phase,
    which frees 64KB/partition through the F/G/top-k phases.
  - PSUM drains alternate DVE/scalar; G-combine partner terms for odd t go
    to gpsimd; top-k(0) chunks are emitted between G(g1) quadrants so the
    DVE queue never blocks the G combines the PE waits on.
  - G accumulates the odd-k tiles first so their PSUM banks drain to SBUF
    while the even-k tiles accumulate, keeping all 8 banks cycling.

Sharding: data-parallel over batch B=8 -> one batch element per NeuronCore.
"""
import numpy as np

B, L, D, H, Dh = 8, 2048, 1024, 16, 64
KTOP = 7
NCORES = 8
P = 128

_prog_cache = {}


# ---------------------------------------------------------------- host helpers
def _round_fp32r(x):
    """Round fp32 to the ~13-bit-mantissa fp32r grid (matches measured HW)."""
    b = np.ascontiguousarray(x, dtype=np.float32).view(np.uint32)
    half = np.uint32(1 << 9)
    keep = np.uint32(0xFFFFFFFF) ^ np.uint32((1 << 10) - 1)
    out = ((b + half) & keep).view(np.float32)
    return np.where(np.isfinite(out), out, 0.0).astype(np.float32)


def _k_of(j, r):
    """k-index of packed row/col 128*j+r in the parity-permuted layout."""
    if j < 4:
        return 2 * (128 * j + r), "re"
    if j < 8:
        return 2 * (128 * (j - 4) + r) + 1, "re"
    if j < 12:
        return 2 * (128 * (j - 8) + r), "im"
    return 2 * (128 * (j - 12) + r) + 1, "im"


def _build_FG():
    """Radix-2-folded DFT matrices in the parity-permuted packed layout.

    Fh (1024, 2048): cols for even-k tiles multiply Qe = q[:1024]+q[1024:],
    odd-k tiles multiply Qo = q[:1024]-q[1024:]; sqrt(c_k/L) folded in.
    G (2048, 1024): rows give n in [0,1024); corr[n+1024] is recovered from
    the even-tile/odd-tile partial sums as A - B. Tile 8 row 0 carries the
    nyquist term (its natural occupant, im k=0, is identically zero)."""
    m = np.arange(1024)[:, None].astype(np.float64)
    n = np.arange(1024)[None, :].astype(np.float64)
    Fh = np.zeros((1024, 2048), dtype=np.float64)
    G = np.zeros((2048, 1024), dtype=np.float64)
    for j in range(16):
        for r in range(128):
            k, ri = _k_of(j, r)
            col = 128 * j + r
            if j == 8 and r == 0:  # nyquist slot
                Fh[:, col] = (np.cos(np.pi * m[:, 0]) * np.sqrt(1.0 / L))
                G[col, :] = np.cos(np.pi * n[0])
                continue
            ck = 1.0 if k == 0 else 2.0
            if ri == "re":
                Fh[:, col] = np.cos(2 * np.pi * m[:, 0] * k / L) * np.sqrt(ck / L)
                G[col, :] = np.cos(2 * np.pi * k * n[0] / L)
            else:
                Fh[:, col] = -np.sin(2 * np.pi * m[:, 0] * k / L) * np.sqrt(ck / L)
                G[col, :] = -np.sin(2 * np.pi * k * n[0] / L)
    return Fh.astype(np.float32), G.astype(np.float32)


def _tile_w(w):
    """(1024, 1024) -> (128, 8, 1024): [p, o, c] = w[o*128+p, c]."""
    return np.ascontiguousarray(
        w.reshape(8, P, D).transpose(1, 0, 2)).astype(np.float32)


def _host_constants():
    Fh, G = _build_FG()
    # fmat[t, p, mt] = [Fh[mt*P+p, t-block] | Fh[mt*P+p, (8+t)-block]]
    # (p-major so the per-t DMA into a [128, 8, 256] tile is contiguous)
    fmat = np.zeros((8, P, 8, 2 * P), dtype=np.float32)
    for t in range(8):
        for mt in range(8):
            fmat[t, :, mt, 0:P] = Fh[mt * P:(mt + 1) * P, t * P:(t + 1) * P]
            fmat[t, :, mt, P:2 * P] = Fh[mt * P:(mt + 1) * P,
                                         (8 + t) * P:(9 + t) * P]
    # gmat[kt, nn] = G[kt*128:(kt+1)*128, nn*512:(nn+1)*512], nn in 0..1
    gmat = np.zeros((16, 2, P, 512), dtype=np.float32)
    for kt in range(16):
        for nn in range(2):
            gmat[kt, nn] = G[kt * P:(kt + 1) * P, nn * 512:(nn + 1) * 512]
    return _round_fp32r(fmat), _round_fp32r(gmat)


# ---------------------------------------------------------------- wait splitting
def _split_excess_waits(nc, mybir):
    """walrus here allows max ONE sem wait per instruction; hoist extras onto
    injected NoOps on the same engine."""
    ctr = 0
    for f in nc.m.functions:
        for bblk in f.blocks:
            insts = bblk.instructions
            i = 0
            while i < len(insts):
                ins = insts[i]
                si = ins.sync_info
                if si is not None and len(si.on_wait) > 1:
                    waits = list(si.on_wait)
                    for w in waits[:-1]:
                        nop = mybir.InstNoOp(name=f"WSPL-{ctr}", ins=[], outs=[])
                        ctr += 1
                        nop.engine = ins.engine
                        nop.sync_info = mybir.SyncInfo(on_wait=[w], on_update=[])
                        insts.insert(i, nop)
                        i += 1
                    ins.sync_info = mybir.SyncInfo(
                        on_wait=[waits[-1]], on_update=list(si.on_update))
                i += 1


# ---------------------------------------------------------------- bass program
def _build_program():
    import concourse.bass as bass
    import concourse.mybir as mybir
    from concourse.tile import TileContext
    from concourse.masks import make_identity

    f32 = mybir.dt.float32
    f32r = mybir.dt.float32r
    i32 = mybir.dt.int32
    u32 = mybir.dt.uint32
    Alu = mybir.AluOpType

    nc = bass.Bass("TRN2", target_bir_lowering=False)

    qin = nc.dram_tensor("qin", (L, D), f32, kind="ExternalInput")
    kin = nc.dram_tensor("kin", (L, D), f32, kind="ExternalInput")
    vin = nc.dram_tensor("vin", (L, D), f32, kind="ExternalInput")
    wq_d = nc.dram_tensor("wq_t", (P, 8, D), f32r, kind="ExternalInput")
    wk_d = nc.dram_tensor("wk_t", (P, 8, D), f32r, kind="ExternalInput")
    wv_d = nc.dram_tensor("wv_t", (P, 8, D), f32r, kind="ExternalInput")
    wo_d = nc.dram_tensor("wo_t", (P, 8, D), f32r, kind="ExternalInput")
    f_d = nc.dram_tensor("fmat", (8, P, 8, 2 * P), f32r, kind="ExternalInput")
    g_d = nc.dram_tensor("gmat", (16, 2, P, 512), f32r, kind="ExternalInput")
    out_d = nc.dram_tensor("out", (L, D), f32, kind="ExternalOutput")
    vp_d = nc.dram_tensor("vproj_dram", (P, 8, L), f32r, kind="Internal")
    taps = {}
    if _prog_cache.get("debug_taps"):
        taps["feq"] = nc.dram_tensor("tap_feq", (2, P, 8, 512), f32r,
                                     kind="ExternalOutput")
        taps["foq"] = nc.dram_tensor("tap_foq", (2, P, 8, 512), f32r,
                                     kind="ExternalOutput")
        taps["sq"] = nc.dram_tensor("tap_sq", (2, P, 16, 512), f32r,
                                    kind="ExternalOutput")
        taps["sp"] = nc.dram_tensor("tap_sp", (2, P, 16, 512), f32r,
                                    kind="ExternalOutput")
        taps["corr"] = nc.dram_tensor("tap_corr", (2, P, 4, L), f32,
                                      kind="ExternalOutput")
        taps["m"] = nc.dram_tensor("tap_m", (P, 8, 64), f32,
                                   kind="ExternalOutput")
        taps["vp"] = nc.dram_tensor("tap_vp", (P, 8, L), f32,
                                    kind="ExternalOutput")
        taps["r2t"] = nc.dram_tensor("tap_r2t", (P, 8, 64, 32), f32r,
                                     kind="ExternalOutput")

    _rr = [0]

    def drain(dst, src_ap):
        # PSUM -> SBUF copy. f32r destinations must go through the DVE
        # (scalar/activation f32r output is not trustworthy); plain f32
        # copies alternate DVE/scalar to spread load.
        if _rr[0] % 2 == 0:
            nc.vector.tensor_copy(dst, src_ap)
        else:
            nc.scalar.copy(dst, src_ap)
        _rr[0] += 1

    with TileContext(nc) as tc:
        with tc.tile_pool(name="const", bufs=1) as cp:
            ident = cp.tile([P, P], f32)
            make_identity(nc, ident)
            # (128, 64) with 1 at (p, p % 64): a 64x64 identity at base 0 or 64
            ident2 = cp.tile([P, 64], f32)
            nc.gpsimd.memset(ident2, 0.0)
            nc.gpsimd.affine_select(
                out=ident2, in_=ident2, compare_op=Alu.not_equal, fill=1.0,
                base=0, channel_multiplier=1, pattern=[[-1, 64]])
            nc.gpsimd.affine_select(
                out=ident2, in_=ident2, compare_op=Alu.not_equal, fill=1.0,
                base=-64, channel_multiplier=1, pattern=[[-1, 64]])
            # T64f[p, s] = (p - s) & 63 as fp32
            t64i = cp.tile([P, 64], i32)
            nc.gpsimd.iota(t64i, pattern=[[-1, 64]], base=0, channel_multiplier=1)
            nc.vector.tensor_scalar(t64i, t64i, 63, None, op0=Alu.bitwise_and)
            t64f = cp.tile([P, 64], f32)
            nc.vector.tensor_copy(t64f, t64i)
            m_all = cp.tile([P, 8, 64], f32)    # roll matrices, (c, s)
            mp_all = cp.tile([P, 8, 64], f32r)  # transposed,  (s, t) per head
            smf = cp.tile([P, 8, 32], f32)      # per-cc top-k scratch
            smu = cp.tile([P, 8, 16], u32)
            tmp64a = cp.tile([P, 64], f32)
            tmp64b = cp.tile([P, 64], f32)

            # ------------- phase P/F: per tensor: fold -> transpose ->
            # project -> spectra.  Q spectra land in s_g; K spectra stay in
            # PSUM and the spectral product overwrites s_g in place.
            def proj_tensor(name, src_d, w_d):
                fep = tc.alloc_tile_pool(name=f"fe{name}", bufs=1)
                fe = [fep.tile([P, 8, 512], f32r, name=f"fe{name}{g}")
                      for g in range(2)]
                fo = [fep.tile([P, 8, 512], f32r, name=f"fo{name}{g}")
                      for g in range(2)]
                wpool = tc.alloc_tile_pool(name=f"w{name}", bufs=1)
                w_sb = wpool.tile([P, 8, D], f32r, name=f"w_{name}")
                for cc in range(8):
                    nc.sync.dma_start(w_sb[:, cc, :], w_d.ap()[:, cc, :])
                with tc.tile_pool(name=f"raw{name}", bufs=2) as rawp, \
                     tc.tile_pool(name=f"eo{name}", bufs=3) as eop, \
                     tc.tile_pool(name=f"xt{name}", bufs=3) as xtp, \
                     tc.tile_pool(name=f"trp{name}", bufs=2,
                                  space="PSUM") as trpp, \
                     tc.tile_pool(name=f"pp{name}", bufs=3,
                                  space="PSUM") as prjp:
                    for mc in range(8):
                        rlo = rawp.tile([P, D], f32, tag="rlo")
                        rhi = rawp.tile([P, D], f32, tag="rhi")
                        nc.sync.dma_start(rlo, src_d.ap()[mc * P:(mc + 1) * P, :])
                        nc.sync.dma_start(
                            rhi, src_d.ap()[(mc + 8) * P:(mc + 9) * P, :])
                        for br, op in ((0, Alu.add), (1, Alu.subtract)):
                            eo = eop.tile([P, D], f32, tag="eo",
                                          name=f"eo_{name}{mc}{br}")
                            nc.vector.tensor_tensor(eo, rlo, rhi, op)
                            # transpose eo -> xt (c-part, m) then project
                            xt = xtp.tile([P, 8, P], f32r, tag="xt",
                                          name=f"xt_{name}{mc}{br}")
                            for half in range(2):
                                trp = trpp.tile([P, 512], f32, tag="tr")
                                for j in range(4):
                                    ct = half * 4 + j
                                    nc.tensor.transpose(
                                        trp[:, j * P:(j + 1) * P],
                                        eo[:, ct * P:(ct + 1) * P], ident)
                                drain(xt[:, half * 4:(half + 1) * 4, :], trp)
                            dst = fe if br == 0 else fo
                            pr = prjp.tile([P, 2, 512], f32, tag="pr",
                                           name=f"pr_{name}{mc}{br}")
                            for g in range(2):
                                for cc in range(8):
                                    nc.tensor.matmul(
                                        pr[:, g, :], xt[:, cc, :],
                                        w_sb[:, cc, g * 512:(g + 1) * 512],
                                        start=(cc == 0), stop=(cc == 7))
                                drain(dst[g][:, mc, :], pr[:, g, :])
                wpool.release()
                return fep, fe, fo

            # ------------- phase V first (independent; warms the PE);
            # projected V^T goes to DRAM and is re-read at the roll phase
            wv_pool = tc.alloc_tile_pool(name="wv", bufs=1)
            wv_sb = wv_pool.tile([P, 8, D], f32r)
            with tc.tile_pool(name="vraw", bufs=3) as vrawp, \
                 tc.tile_pool(name="vxt", bufs=2) as vxtp, \
                 tc.tile_pool(name="vst", bufs=6) as vstp, \
                 tc.tile_pool(name="vtrp", bufs=2, space="PSUM") as vtrpp, \
                 tc.tile_pool(name="vpp", bufs=6, space="PSUM") as vpp:
                vxts = [None] * 4

                def v_transpose(mb):
                    vxt = vxtp.tile([P, 8, 512], f32r, tag="vxt",
                                    name=f"vxt_{mb}")
                    vxts[mb] = vxt
                    for mc in range(4):
                        raw = vrawp.tile([P, D], f32, tag="vraw")
                        nc.sync.dma_start(
                            raw, vin.ap()[(mb * 4 + mc) * P:
                                          (mb * 4 + mc + 1) * P, :])
                        for half in range(2):
                            trp = vtrpp.tile([P, 512], f32, tag="vtr")
                            for j in range(4):
                                ct = half * 4 + j
                                nc.tensor.transpose(
                                    trp[:, j * P:(j + 1) * P],
                                    raw[:, ct * P:(ct + 1) * P], ident)
                            drain(vxt[:, half * 4:(half + 1) * 4,
                                      mc * P:(mc + 1) * P], trp)

                def v_project(mb):
                    vxt = vxts[mb]
                    for cc8 in range(8):
                        pv = vpp.tile([P, 512], f32, tag="vpr")
                        for ct in range(8):
                            nc.tensor.matmul(
                                pv, wv_sb[:, ct, cc8 * P:(cc8 + 1) * P],
                                vxt[:, ct, :],
                                start=(ct == 0), stop=(ct == 7))
                        vst = vstp.tile([P, 512], f32r, tag="vst")
                        drain(vst, pv)
                        nc.sync.dma_start(
                            vp_d.ap()[:, cc8, mb * 512:(mb + 1) * 512], vst)

                v_transpose(0)
                for cc in range(8):
                    nc.sync.dma_start(wv_sb[:, cc, :], wv_d.ap()[:, cc, :])
                v_project(0)
                for mb in range(1, 4):
                    v_transpose(mb)
                    v_project(mb)
            wv_pool.release()

            s_pool = tc.alloc_tile_pool(name="sg", bufs=1)
            s_g = [s_pool.tile([P, 16, 512], f32r, name=f"s{g}")
                   for g in range(2)]

            # ---- Q: project + spectra into s_g
            fep_q, feq, foq = proj_tensor("q", qin, wq_d)
            with tc.tile_pool(name="ftiq", bufs=4) as ftp, \
                 tc.tile_pool(name="fpsq", bufs=4, space="PSUM") as fps:
                for g in range(2):
                    for t in range(8):
                        fti = ftp.tile([P, 8, 2 * P], f32r, tag="fti",
                                       name=f"ftiq_{g}_{t}")
                        nc.sync.dma_start(fti, f_d.ap()[t])
                        rhs_q = (feq if t < 4 else foq)[g]
                        pq = fps.tile([P, 2, 512], f32, tag="pq",
                                      name=f"pq_{g}_{t}")
                        for mt in range(8):
                            st, sp = (mt == 0), (mt == 7)
                            nc.tensor.matmul(pq[:, 0, :], fti[:, mt, 0:P],
                                             rhs_q[:, mt, :], start=st, stop=sp)
                            nc.tensor.matmul(pq[:, 1, :], fti[:, mt, P:2 * P],
                                             rhs_q[:, mt, :], start=st, stop=sp)
                        drain(s_g[g][:, t, :], pq[:, 0, :])
                        drain(s_g[g][:, 8 + t, :], pq[:, 1, :])
            if taps:
                for g in range(2):
                    nc.sync.dma_start(taps["feq"].ap()[g], feq[g])
                    nc.sync.dma_start(taps["foq"].ap()[g], foq[g])
                    nc.sync.dma_start(taps["sq"].ap()[g], s_g[g])
            fep_q.release()

            # ---- K: project + spectra; product overwrites s_g in place
            fep_k, fek, fok = proj_tensor("k", kin, wk_d)
            with tc.tile_pool(name="ftik", bufs=4) as ftp, \
                 tc.tile_pool(name="sppk", bufs=3) as spp, \
                 tc.tile_pool(name="fpsk", bufs=3, space="PSUM") as fps:
                for g in range(2):
                    for t in range(8):
                        fti = ftp.tile([P, 8, 2 * P], f32r, tag="fti",
                                       name=f"ftik_{g}_{t}")
                        nc.sync.dma_start(fti, f_d.ap()[t])
                        rhs_k = (fek if t < 4 else fok)[g]
                        pk = fps.tile([P, 2, 512], f32, tag="pk",
                                      name=f"pk_{g}_{t}")
                        for mt in range(8):
                            st, sp = (mt == 0), (mt == 7)
                            nc.tensor.matmul(pk[:, 0, :], fti[:, mt, 0:P],
                                             rhs_k[:, mt, :], start=st, stop=sp)
                            nc.tensor.matmul(pk[:, 1, :], fti[:, mt, P:2 * P],
                                             rhs_k[:, mt, :], start=st, stop=sp)
                        # spectral product: s_re = qre*kre + qim*kim,
                        # s_im = qim*kre - qre*kim (in-place over q spectra)
                        qre = s_g[g][:, t, :]
                        qim = s_g[g][:, 8 + t, :]
                        kre = pk[:, 0, :]
                        kim = pk[:, 1, :]
                        tm1 = spp.tile([P, 512], f32, tag="tm1")
                        tm2 = spp.tile([P, 512], f32, tag="tm2")
                        tm3 = spp.tile([P, 512], f32, tag="tm3")
                        tm4 = spp.tile([P, 512], f32, tag="tm4")
                        if t == 0:
                            # row 0 of tile 0 = DC (qre*kre); row 0 of tile 8
                            # = nyquist (qim*kim); compute before overwrite
                            dcny = spp.tile([1, 1024], f32, tag="dc",
                                            name=f"dcny_{g}")
                            nc.vector.tensor_tensor(
                                dcny[0:1, 0:512], qre[0:1, :], kre[0:1, :],
                                Alu.mult)
                            nc.vector.tensor_tensor(
                                dcny[0:1, 512:1024], qim[0:1, :], kim[0:1, :],
                                Alu.mult)
                        nc.vector.tensor_tensor(tm1, kre, qre, Alu.mult)
                        nc.vector.tensor_tensor(tm2, kim, qim, Alu.mult)
                        nc.vector.tensor_tensor(tm3, kre, qim, Alu.mult)
                        nc.vector.tensor_tensor(tm4, kim, qre, Alu.mult)
                        ceng = nc.vector if t % 2 == 0 else nc.gpsimd
                        ceng.tensor_tensor(qre, tm1, tm2, Alu.add)
                        ceng.tensor_tensor(qim, tm3, tm4, Alu.subtract)
                        if t == 0:
                            nc.vector.tensor_copy(s_g[g][0:1, 0, :],
                                                  dcny[0:1, 0:512])
                            nc.vector.tensor_copy(s_g[g][0:1, 8, :],
                                                  dcny[0:1, 512:1024])
            if taps:
                for g in range(2):
                    nc.sync.dma_start(taps["sp"].ap()[g], s_g[g])
            fep_k.release()

            def topk_chunk(g, c):
                if True:
                    gt_idx = g * 4 + c
                    topv = smf[:, gt_idx, 0:8]
                    expw = smf[:, gt_idx, 8:16]
                    shmf = smf[:, gt_idx, 16:24]
                    nv0 = smf[:, gt_idx, 24:25]
                    s7 = smf[:, gt_idx, 25:26]
                    r7 = smf[:, gt_idx, 26:27]
                    topi = smu[:, gt_idx, 0:8]
                    shmi = smu[:, gt_idx, 8:16]
                    nc.vector.max(out=topv, in_=corr[g][:, c, :])
                    nc.vector.max_index(out=topi, in_max=topv,
                                        in_values=corr[g][:, c, :])
                    nc.vector.tensor_scalar(nv0, topv[:, 0:1], -1.0, None,
                                            op0=Alu.mult)
                    nc.scalar.activation(
                        expw[:, 0:KTOP], topv[:, 0:KTOP],
                        mybir.ActivationFunctionType.Exp,
                        bias=nv0, scale=1.0)
                    nc.vector.reduce_sum(s7, expw[:, 0:KTOP],
                                         axis=mybir.AxisListType.X)
                    nc.vector.reciprocal(r7, s7)
                    nc.vector.tensor_scalar(expw[:, 0:KTOP], expw[:, 0:KTOP],
                                            r7, None, op0=Alu.mult)
                    nc.vector.tensor_scalar(shmi, topi, 63, None,
                                            op0=Alu.bitwise_and)
                    nc.vector.tensor_copy(shmf, shmi)
                    tmp64 = tmp64a if (c % 2 == 0) else tmp64b
                    for i in range(KTOP):
                        dst = m_all[:, gt_idx, :] if i == 0 else tmp64
                        nc.vector.tensor_scalar(
                            dst, t64f, shmf[:, i:i + 1], expw[:, i:i + 1],
                            op0=Alu.is_equal, op1=Alu.mult)
                        if i > 0:
                            nc.vector.tensor_tensor(
                                m_all[:, gt_idx, :], m_all[:, gt_idx, :],
                                tmp64, Alu.add)

            # ------------- phase G + top-k(0) interleaved
            corr_pool = tc.alloc_tile_pool(name="corr", bufs=1)
            corr = [corr_pool.tile([P, 4, L], f32, name=f"corr{g}")
                    for g in range(2)]

            ODD = (4, 5, 6, 7, 12, 13, 14, 15)
            EVEN = (0, 1, 2, 3, 8, 9, 10, 11)
            with tc.tile_pool(name="gt", bufs=8) as gtp, \
                 tc.tile_pool(name="pbs", bufs=1) as pbsp, \
                 tc.tile_pool(name="gps", bufs=1, space="PSUM") as gps:
                def g_quadrant(g, nn):
                    pB = [gps.tile([P, 512], f32, tag=f"B{c}",
                                   name=f"pB_{g}_{nn}_{c}")
                          for c in range(4)]
                    for kt in ODD:
                        gt = gtp.tile([P, 512], f32r, tag="gt",
                                      name=f"gt_{g}_{nn}_{kt}")
                        nc.sync.dma_start(gt, g_d.ap()[kt, nn])
                        for c in range(4):
                            nc.tensor.matmul(
                                pB[c], s_g[g][:, kt, c * P:(c + 1) * P],
                                gt, start=(kt == 4), stop=(kt == 15))
                    pBs = [pbsp.tile([P, 512], f32, tag=f"Bs{c}",
                                     name=f"pBs_{g}_{nn}_{c}")
                           for c in range(4)]
                    for c in range(4):
                        nc.scalar.copy(pBs[c], pB[c])
                    pA = [gps.tile([P, 512], f32, tag=f"A{c}",
                                   name=f"pA_{g}_{nn}_{c}")
                          for c in range(4)]
                    for kt in EVEN:
                        gt = gtp.tile([P, 512], f32r, tag="gt",
                                      name=f"gt_{g}_{nn}_{kt}")
                        nc.sync.dma_start(gt, g_d.ap()[kt, nn])
                        for c in range(4):
                            nc.tensor.matmul(
                                pA[c], s_g[g][:, kt, c * P:(c + 1) * P],
                                gt, start=(kt == 0), stop=(kt == 11))
                    for c in range(4):
                        nc.vector.tensor_tensor(
                            corr[g][:, c, nn * 512:(nn + 1) * 512],
                            pA[c], pBs[c], Alu.add)
                        nc.vector.tensor_tensor(
                            corr[g][:, c, 1024 + nn * 512:
                                    1024 + (nn + 1) * 512],
                            pA[c], pBs[c], Alu.subtract)

                g_quadrant(0, 0)
                g_quadrant(0, 1)
                g_quadrant(1, 0)
                topk_chunk(0, 0)
                topk_chunk(0, 1)
                g_quadrant(1, 1)
                topk_chunk(0, 2)
                topk_chunk(0, 3)

            if taps:
                for g in range(2):
                    nc.sync.dma_start(taps["corr"].ap()[g], corr[g])
            for c in range(4):
                topk_chunk(1, c)
            if taps:
                nc.sync.dma_start(taps["m"].ap(), m_all)
            corr_pool.release()
            s_pool.release()

            # ------------- roll + scramble + final projection
            wo_pool = tc.alloc_tile_pool(name="wo", bufs=1)
            wo_sb = wo_pool.tile([P, 8, D], f32r)
            for cc in range(8):
                nc.sync.dma_start(wo_sb[:, cc, :], wo_d.ap()[:, cc, :])
            r2t_pool = tc.alloc_tile_pool(name="r2t", bufs=1)
            r2t = r2t_pool.tile([P, 8, 64, 32], f32r)   # (p, kc, d, hp)

            with tc.tile_pool(name="vsin", bufs=3) as vsinp, \
                 tc.tile_pool(name="mtp", bufs=2, space="PSUM") as mtp, \
                 tc.tile_pool(name="rollp", bufs=6, space="PSUM") as rop:
                for h2 in range(8):
                    vsin = vsinp.tile([P, L], f32r, tag="vsin",
                                      name=f"vsin_{h2}")
                    nc.sync.dma_start(vsin, vp_d.ap()[:, h2, :])
                    for hh in range(2):
                        h = 2 * h2 + hh
                        pb = (h % 2) * 64
                        pm = mtp.tile([64, 64], f32, tag="mt")
                        nc.tensor.transpose(
                            pm, m_all[pb:pb + 64, h // 2, :],
                            ident2[pb:pb + 64, :])
                        nc.scalar.copy(mp_all[pb:pb + 64, h // 2, :], pm)
                        for kc in range(8):
                            pr = rop.tile([P, 2, 64], f32, tag="roll",
                                          name=f"pr_{h}_{kc}")
                            for pp_ in range(2):
                                lc = kc + 8 * pp_
                                nc.tensor.matmul(
                                    pr[:, pp_, :],
                                    vsin[pb:pb + 64, lc * P:(lc + 1) * P],
                                    mp_all[pb:pb + 64, h // 2, :],
                                    start=True, stop=True)
                            # heads 0-7: scalar only (DVE still on top-k);
                            # heads 8-15: alternate DVE/scalar
                            if h < 8 or (h * 8 + kc) % 2 == 1:
                                nc.scalar.copy(
                                    r2t[:, kc, :, h * 2:h * 2 + 2],
                                    pr.rearrange("p j d -> p d j"))
                            else:
                                nc.vector.tensor_copy(
                                    r2t[:, kc, :, h * 2:h * 2 + 2],
                                    pr.rearrange("p j d -> p d j"))
            if taps:
                nc.sync.dma_start(taps["r2t"].ap(), r2t)
            with tc.tile_pool(name="fpp", bufs=6, space="PSUM") as fpp, \
                 tc.tile_pool(name="osb", bufs=6) as osbp:
                for a in range(16):
                    for j2 in range(2):
                        pf = fpp.tile([P, 512], f32, tag="fin")
                        for kc in range(8):
                            nc.tensor.matmul(
                                pf, r2t[:, kc, 4 * a:4 * a + 4, :],
                                wo_sb[:, kc, j2 * 512:(j2 + 1) * 512],
                                start=(kc == 0), stop=(kc == 7))
                        osb = osbp.tile([P, 512], f32, tag="osb")
                        drain(osb, pf)
                        nc.sync.dma_start(
                            out_d.ap()[a * P:(a + 1) * P,
                                       j2 * 512:(j2 + 1) * 512], osb)
            r2t_pool.release()
            wo_pool.release()

    _split_excess_waits(nc, mybir)
    return nc


def _get_program():
    if "nc" not in _prog_cache:
        _prog_cache["nc"] = _build_program()
    return _prog_cache["nc"]


# ---------------------------------------------------------------- entry point
def _last_in_maps_get():
    return _prog_cache["last_in_maps"]


def kernel(queries, keys, values, wq, wk, wv, wo):
    from concourse.bass_utils import run_bass_kernel_spmd

    queries = np.ascontiguousarray(queries, np.float32)
    keys = np.ascontiguousarray(keys, np.float32)
    values = np.ascontiguousarray(values, np.float32)

    if "fg" not in _prog_cache:
        _prog_cache["fg"] = _host_constants()
    fmat, gmat = _prog_cache["fg"]
    consts = {
        "fmat": fmat, "gmat": gmat,
        "wq_t": _round_fp32r(_tile_w(np.asarray(wq, np.float32))),
        "wk_t": _round_fp32r(_tile_w(np.asarray(wk, np.float32))),
        "wv_t": _round_fp32r(_tile_w(np.asarray(wv, np.float32))),
        "wo_t": _round_fp32r(_tile_w(np.asarray(wo, np.float32))),
    }

    nc = _get_program()
    in_maps = []
    for b in range(NCORES):
        in_maps.append({
            "qin": np.ascontiguousarray(queries[b]),
            "kin": np.ascontiguousarray(keys[b]),
            "vin": np.ascontiguousarray(values[b]),
            **consts,
        })
    _prog_cache["last_in_maps"] = in_maps
    res = run_bass_kernel_spmd(nc, in_maps, core_ids=list(range(NCORES)),
                               trace=False)
    out = np.stack([res.results[b]["out"] for b in range(NCORES)], axis=0)
    return out.astype(np.float32)
